# revision 49
# baseline (speedup 1.0000x reference)
"""BlanchotianAttention TRN2 kernel: 8 NeuronCores, data-parallel over batch (2)
x tensor-parallel over heads (4 heads/core).

v3 over the 260us baseline:
  - scores matmuls in fp8e4 DoubleRow perf mode (0.5 cyc/row): Q and K are
    quantized to fp8 during the stage-A PSUM evacuation copies. The DoubleRow
    second k-tile slot is zero-filled (contraction depth is only d=64). The
    dim^-0.5/temp score scale is split between q (x0.25) and k (x 4*scale_h)
    so both fp8 operands sit in e4m3's normal range.
  - inputs DMA directly into per-chunk float32r SBUF tiles (f32r is
    bit-identical to f32; rounding happens inside the PE). Per-chunk tiles
    matter: the Tile framework tracks dependencies per tile, so one big tile
    would make the first matmul wait for every input DMA.
  - DMA arrival order is tuned for the serial DMA pipe: interleaved
    (w_qk[ko], xT[ko, sc0]) pairs feed the first stage-A accumulation at pipe
    speed; later chunks stream behind. Stage A runs ko-major so accumulation
    tracks DMA arrival.
  - first exp fires at ~15us (vs ~44us) and ACT (the bottleneck engine at
    ~142us busy) runs with few gaps; tail norm/outproj is split across
    DVE+GPSIMD.

Layout (per core, batch b, head-group hg = heads h0..h0+3):
  - stage A-qk: psum = w.T @ xT -> QT/KT [d, seq] head-pair tiles, evacuated
    to fp8 qt8/kt8 (partitions 0:64 even head of pair, 64:128 odd head).
  - stage A-v: V = x @ w_v in [seq, d] layout, augmented per head with a ones
    block: V_aug[j, h*128 : h*128+128] = [v_h (64) | ones (64)] (f32r).
  - stage B/C per (512-wide i-chunk, head pair, 128-wide j-tile):
    S^T = K @ Q^T via one DoubleRow matmul per head,
    P = exp(S^T) on ACT (one [128,1024] activation covers both heads),
    PV+l fused f32r: matmul(lhsT=[v_h | ones], rhs=P) accumulates attn@v in
    PSUM rows 0-63 and the softmax denominator (broadcast) in rows 64-127.
  - void token: the void QUERY row is dropped by the reference so it is never
    computed. The void KEY/VALUE occupy j=2048 inside j-tile 16, zero-padded
    to 128 rows; a per-partition exp bias of -100 on that tile zeroes the pad
    rows' contributions.
  - normalize: reciprocal of the l rows + cross-base multiply -> O^T pair tile.
  - stage D: y_partial = O_norm @ w_out_shard; host sums partials over the 4
    head-group cores of each batch (+ b_out).
"""
import sys

sys.path.insert(0, "/opt/trn_rl_repo")

import numpy as np

DIM, HEADS, B, N = 1024, 16, 2, 2048
D = DIM // HEADS          # 64
HPC = HEADS // 4          # heads per core = 4
NJT = 17                  # j tiles (16 full + void/pad tile)
P = 128

_cache = {}
DEBUG = False


def _build():
    import concourse.bass as bass
    import concourse.mybir as mybir
    import concourse.tile as tile
    from concourse import bacc

    F32 = mybir.dt.float32
    F32R = mybir.dt.float32r
    BF16 = mybir.dt.bfloat16
    FP8 = mybir.dt.float8e4
    DR = mybir.MatmulPerfMode.DoubleRow
    Exp = mybir.ActivationFunctionType.Exp
    Rcp = mybir.ActivationFunctionType.Reciprocal

    nc = bacc.Bacc("TRN2", target_bir_lowering=False, debug=False)
    xT = nc.dram_tensor("xT", [DIM, N], F32R, kind="ExternalInput").ap()
    wqkv = nc.dram_tensor("wqkv", [DIM, 768], F32R, kind="ExternalInput").ap()
    wout = nc.dram_tensor("wout", [256, DIM], F32R, kind="ExternalInput").ap()
    voidk = nc.dram_tensor("voidk", [P, 2], F32, kind="ExternalInput").ap()
    voidv = nc.dram_tensor("voidv", [1, 256], F32, kind="ExternalInput").ap()
    ebias_in = nc.dram_tensor("ebias_in", [P, 1], F32, kind="ExternalInput").ap()
    y = nc.dram_tensor("y", [N, DIM], F32, kind="ExternalOutput").ap()
    if DEBUG:
        dbg_qt = nc.dram_tensor("dbg_qt", [P, 2, 2, N], FP8,
                                kind="ExternalOutput").ap()
        dbg_kt = nc.dram_tensor("dbg_kt", [P, 2, 2, NJT * P], FP8,
                                kind="ExternalOutput").ap()
        dbg_va = nc.dram_tensor("dbg_va", [P, NJT, 512], BF16,
                                kind="ExternalOutput").ap()
        dbg_pvl = nc.dram_tensor("dbg_pvl", [P, 512], F32,
                                 kind="ExternalOutput").ap()

    KO = DIM // P  # 8 k-tiles

    with tile.TileContext(nc) as tc:
        with tc.tile_pool(name="persist", bufs=1) as pp, \
             tc.tile_pool(name="work", bufs=1) as wp, \
             tc.tile_pool(name="psum", bufs=1, space="PSUM") as ps, \
             tc.tile_pool(name="loadA", bufs=2) as lp:

            # ---- constants ----
            ones = pp.tile([P, D], F32)
            nc.vector.memset(ones[:], 1.0)
            ebias = pp.tile([P, 1], F32)

            # ---- persistent SBUF tensors ----
            # qt8/kt8: [part, pair, doublerow-ktile, seq]; partitions 0:64 =
            # even head of the pair, 64:128 = odd head; ktile slot 1 is zeroed.
            qt8 = pp.tile([P, 2, 2, N], FP8)
            kt8 = pp.tile([P, 2, 2, NJT * P], FP8)
            va = pp.tile([P, NJT, 512], BF16)          # V_aug per j-tile
            wq_t = [pp.tile([P, 512], F32R, name=f"wq_{ko}") for ko in range(KO)]
            wv_t = [pp.tile([P, 256], F32R, name=f"wv_{ko}") for ko in range(KO)]
            wout_t = [pp.tile([P, DIM], F32R, name=f"wout_{h}") for h in range(2)]
            xt_t = [[pp.tile([P, 512], F32R, name=f"xt_{ko}_{sc}")
                     for sc in range(4)] for ko in range(KO)]

            # DoubleRow slot-1 zeros: off the DMA critical path
            nc.vector.memset(qt8[:, :, 1, :], 0.0)
            nc.gpsimd.memset(kt8[:, :, 1, :], 0.0)

            # V_aug ones blocks (DVE, depends only on `ones`)
            for jt in range(NJT):
                nc.vector.tensor_copy(
                    va[:, jt, :].rearrange("p (h c) -> p h c", c=P)[:, :, D:P],
                    ones[:, None, :].to_broadcast([P, 4, D]))

            # ---- input DMA: serial DMA pipe => arrival order is everything.
            # Nothing on the scalar (ACT) queue: DMA issue instructions with
            # ring-full waits clog ACT.SEQ and stall the exps behind them.
            for ko in range(KO):
                nc.sync.dma_start(wq_t[ko][:], wqkv[ko * P:(ko + 1) * P, 0:512])
                nc.sync.dma_start(xt_t[ko][0][:],
                                  xT[ko * P:(ko + 1) * P, 0:512])
            for ko in range(KO):
                nc.sync.dma_start(xt_t[ko][1][:],
                                  xT[ko * P:(ko + 1) * P, 512:1024])
            for ko in range(KO):
                nc.sync.dma_start(wv_t[ko][:],
                                  wqkv[ko * P:(ko + 1) * P, 512:768])
            for ko in range(KO):
                nc.sync.dma_start(xt_t[ko][2][:],
                                  xT[ko * P:(ko + 1) * P, 1024:1536])
            nc.sync.dma_start(ebias[:], ebias_in)
            vkt = lp.tile([P, 2], F32, tag="stg", name="vkt")
            nc.sync.dma_start(vkt[:], voidk)
            vvt = lp.tile([1, 256], F32, tag="stg", name="vvt")
            nc.sync.dma_start(vvt[:], voidv)
            for ko in range(KO):
                nc.sync.dma_start(xt_t[ko][3][:],
                                  xT[ko * P:(ko + 1) * P, 1536:2048])
            for h in range(2):
                nc.sync.dma_start(wout_t[h][:], wout[h * P:(h + 1) * P, :])

            # ---- stage A emit helpers ----
            def emit_aqk_pair(sc, fts, tag="srot0"):
                # fts: (0,1) or (2,3), both packed in ONE psum tile (the srot
                # tag ring is the scarce resource); ko-major so PSUM
                # accumulation tracks DMA arrival.
                acc = ps.tile([P, 1024], F32, tag=tag,
                              name=f"aqk_{sc}_{fts[0]}")
                for ko in range(KO):
                    for i, ft in enumerate(fts):
                        nc.tensor.matmul(
                            acc[:, i * 512:(i + 1) * 512],
                            wq_t[ko][:, ft * P:(ft + 1) * P],
                            xt_t[ko][sc][:],
                            start=(ko == 0), stop=(ko == KO - 1),
                        )
                for i, ft in enumerate(fts):
                    dst = qt8 if ft < 2 else kt8
                    nc.vector.tensor_copy(
                        dst[:, ft % 2, 0, sc * 512:(sc + 1) * 512],
                        acc[:, i * 512:(i + 1) * 512])

            def emit_av2(q, half, tag):
                # 2 avs per psum tile, one per 2KB bank: PSUM start=True
                # resets the whole bank, so two accumulation groups must not
                # share one
                acc = ps.tile([P, 1024], F32, tag=tag, name=f"avw_{q}_{half}")
                offs = (0, 512)
                stis = (2 * half, 2 * half + 1)
                for ko in range(KO):
                    for i, sti in enumerate(stis):
                        nc.tensor.matmul(
                            acc[:, offs[i]:offs[i] + 256],
                            xt_t[ko][q][:, sti * P:(sti + 1) * P],
                            wv_t[ko][:],
                            start=(ko == 0), stop=(ko == KO - 1),
                        )
                for i, sti in enumerate(stis):
                    st = 4 * q + sti
                    nc.vector.tensor_copy(
                        va[:, st, :].rearrange("p (h c) -> p h c", c=P)[:, :, 0:D],
                        acc[:, offs[i]:offs[i] + 256]
                        .rearrange("p (h c) -> p h c", c=D))

            def emit_void_setup():
                # void k column + pad zeros + V_aug void row
                nc.vector.tensor_copy(kt8[:, :, 0, 2048:2049], vkt[:])
                nc.vector.memset(kt8[:, :, 0, 2049:NJT * P], 0.0)
                va16 = va[:, 16, :]
                nc.vector.memset(
                    va16.rearrange("p (h c) -> p h c", c=P)[:, :, 0:D]
                    .bitcast(F32), 0.0)
                nc.vector.tensor_copy(
                    va16.rearrange("p (h c) -> p h c", c=P)[0:1, :, 0:D],
                    vvt[:].rearrange("p (h c) -> p h c", c=D))

            # ---- stage B/C/D emit helpers ----
            def emit_scores_pair(ic, jt, pair):
                isl = slice(ic * 512, (ic + 1) * 512)
                jsl = slice(jt * P, (jt + 1) * P)
                s_pair = ps.tile([P, 1024], F32, tag=f"srot{pair}",
                                 name=f"s_{ic}_{jt}_{pair}")
                for hh in range(2):
                    nc.tensor.matmul(
                        s_pair[:, hh * 512:(hh + 1) * 512],
                        kt8[hh * D:(hh + 1) * D, pair, :, jsl],
                        qt8[hh * D:(hh + 1) * D, pair, :, isl],
                        start=True, stop=True, perf_mode=DR)
                return s_pair

            def emit_scores(ic, jt):
                return [emit_scores_pair(ic, jt, pair) for pair in range(2)]

            def emit_exps(ic, jt, s_cur):
                p_tiles = []
                for pair in range(2):
                    p_pair = wp.tile([P, 1024], BF16, tag=f"pexp{pair}",
                                     bufs=6,
                                     name=f"p_{ic}_{jt}_{pair}")
                    if jt == 16:
                        nc.scalar.activation(p_pair[:], s_cur[pair][:], Exp,
                                             bias=ebias[:])
                    else:
                        nc.scalar.activation(p_pair[:], s_cur[pair][:], Exp)
                    p_tiles.append(p_pair)
                return p_tiles

            def emit_pvl(ic, jt, p_tiles, pvl):
                for pair in range(2):
                    for hh in range(2):
                        h = 2 * pair + hh
                        nc.tensor.matmul(
                            pvl[h][:],
                            va[:, jt, h * P:(h + 1) * P],
                            p_tiles[pair][:, hh * 512:(hh + 1) * 512],
                            start=(jt == 0), stop=(jt == 16),
                        )

            def emit_exp_pvl(ic, jt, s_cur, pvl, nxt, mid=None):
                """exp(jt) ; scores(nxt) ; [mid()] ; pvl(jt)."""
                p_tiles = emit_exps(ic, jt, s_cur)
                s_nxt = emit_scores(*nxt) if nxt is not None else None
                if mid is not None:
                    mid()
                emit_pvl(ic, jt, p_tiles, pvl)
                return s_nxt

            def emit_norm(ic, pvl, final=False):
                """normalize pvl -> osb SBUF tiles."""
                osb = [wp.tile([P, 512], F32R,
                               tag=f"osbf{pair}" if final else f"osb{pair}",
                               bufs=2, name=f"osb{pair}_{ic}")
                       for pair in range(2)]
                rsbs = []
                for h in range(4):
                    r_sb = lp.tile([P, 512], F32, tag="rsbf" if final else "rsb",
                                   bufs=4 if final else 2,
                                   name=f"rsb_{ic}_{h}")
                    rsbs.append(r_sb)
                    nc.vector.reciprocal(r_sb[D:P, :], pvl[h][D:P, :])
                    if not final:
                        pair, hh = divmod(h, 2)
                        nc.vector.tensor_tensor(
                            osb[pair][hh * D:(hh + 1) * D, :],
                            pvl[h][0:D, :], r_sb[D:P, :],
                            mybir.AluOpType.mult)
                if final:
                    # all recips first, then the mults
                    for h in range(4):
                        pair, hh = divmod(h, 2)
                        nc.vector.tensor_tensor(
                            osb[pair][hh * D:(hh + 1) * D, :],
                            pvl[h][0:D, :], rsbs[h][D:P, :],
                            mybir.AluOpType.mult)
                return osb

            def emit_outproj_it(ic, osb, yps, it, final=False):
                # yps are DEAD psum tiles (the drained pvl accumulators / last
                # score tiles) written in place: allocating fresh psum tiles
                # would share slots with the next ic's pvl accumulators via
                # the pool's LIFO allocator and serialize the whole tail
                ysb = wp.tile([P, DIM], F32, tag="ysbf" if final else "ysb",
                              bufs=2 if final else 4,
                              name=f"ysb_{ic}_{it}")
                for oc in range(2):
                    yp = yps[it * 2 + oc]
                    for pair in range(2):
                        nc.tensor.matmul(
                            yp[:],
                            osb[pair][:, it * P:(it + 1) * P],
                            wout_t[pair][:, oc * 512:(oc + 1) * 512],
                            start=(pair == 0), stop=(pair == 1),
                        )
                    if final and oc == 1:
                        # ACT is idle after the last exp and can read PSUM
                        nc.scalar.copy(ysb[:, oc * 512:(oc + 1) * 512], yp[:])
                    else:
                        nc.vector.tensor_copy(ysb[:, oc * 512:(oc + 1) * 512],
                                              yp[:])
                    if final:
                        # don't wait for the other half: stream each 512-col
                        # half as soon as its copy lands
                        nc.sync.dma_start(
                            y[ic * 512 + it * P: ic * 512 + (it + 1) * P,
                              oc * 512:(oc + 1) * 512],
                            ysb[:, oc * 512:(oc + 1) * 512])
                if not final:
                    nc.sync.dma_start(
                        y[ic * 512 + it * P: ic * 512 + (it + 1) * P, :],
                        ysb[:])

            def alloc_pvl(ic):
                return [ps.tile([P, 512], F32, tag=f"pvl{h}", name=f"pvl{h}_{ic}")
                        for h in range(4)]

            # ---- main schedule ----
            # sc0's 4 q/k features run upfront, ko-major (tracking DMA
            # arrival), packed into the two srot tiles' halves so both score
            # pairs can fire back-to-back; ic1..3 plain.
            pvl = alloc_pvl(0)
            accp = [ps.tile([P, 1024], F32, tag=f"srot{i}", name=f"a0_p{i}")
                    for i in range(2)]
            for ko in range(KO):
                # (ft, pair-acc, col offset): pair0 = ft0|ft2, pair1 = ft1|ft3
                for ft, pair in ((0, 0), (2, 0), (1, 1), (3, 1)):
                    off = 0 if ft < 2 else 512
                    nc.tensor.matmul(
                        accp[pair][:, off:off + 512],
                        wq_t[ko][:, ft * P:(ft + 1) * P],
                        xt_t[ko][0][:],
                        start=(ko == 0), stop=(ko == KO - 1),
                    )
            for pair in range(2):
                nc.vector.tensor_copy(qt8[:, pair, 0, 0:512],
                                      accp[pair][:, 0:512])
                nc.vector.tensor_copy(kt8[:, pair, 0, 0:512],
                                      accp[pair][:, 512:1024])
            s00_p0 = emit_scores_pair(0, 0, 0)
            s00_p1 = emit_scores_pair(0, 0, 1)
            s_cur = [s00_p0, s00_p1]
            # jts 0-2: exps + next-scores only; their pvl matmuls are emitted
            # after the avs so the av matmuls (gated on the w_v DMA) never sit
            # between scores in the in-order PE stream
            deferred = []
            for jt in range(0, 3):
                p_tiles = emit_exps(0, jt, s_cur)
                if jt == 1:
                    # av accs take ring slots here so they only wait early
                    # exp reads, keeping pvl(0,*) fed without stalling the
                    # jt3+ score chain
                    emit_av2(0, 0, "srot1")
                if jt == 2:
                    emit_av2(0, 1, "srot1")
                s_cur = emit_scores(0, jt + 1)
                deferred.append((jt, p_tiles))
            for jt, p_tiles in deferred:
                emit_pvl(0, jt, p_tiles, pvl)
            for q in range(1, 4):
                emit_aqk_pair(q, (2, 3), tag="srot0")
                emit_av2(q, 0, "srot1")
                if q == 2:
                    emit_void_setup()
                lo, hi = 4 * q - 1, 4 * q + 3   # jts whose next-scores live in sc q
                for jt in range(lo, hi if q < 3 else NJT):
                    nxt = (0, jt + 1) if jt < NJT - 1 else (1, 0)
                    s_cur = emit_exp_pvl(0, jt, s_cur, pvl, nxt)
                    if jt == lo + 1:
                        emit_av2(q, 1, "srot0")
                    if jt == lo + 2 and q == 1:
                        # sc1 q-features feed ic1's scores (at ic0-jt16)
                        emit_aqk_pair(q, (0, 1), tag="srot1")

            if DEBUG:
                nc.sync.dma_start(dbg_qt, qt8[:])
                nc.sync.dma_start(dbg_kt, kt8[:])
                nc.sync.dma_start(dbg_va, va[:])
                dbg_sb = wp.tile([P, DIM], F32, tag="ysb", bufs=4,
                                 name="dbg_sb")
                nc.vector.tensor_copy(dbg_sb[:, 0:512], pvl[0][:])
                nc.sync.dma_start(dbg_pvl, dbg_sb[:, 0:512])
            pvl_prev = pvl
            s_fin = None
            # outproj(ic-1) writes its y psum in place into the drained
            # pvl(ic-1) tiles; pvl(ic)'s tag slots alias those banks, so its
            # allocation + first jts' matmuls are deferred past the outproj
            # writes (p tiles wait in the pexp ring)
            flush_plan = {5: (0, 1, 2), 6: (3, 4, 5), 7: (6, 7)}
            for ic in range(1, 4):
                osb = emit_norm(ic - 1, pvl_prev)
                yps = [pvl_prev[k % 4] for k in range(8)]
                pvl = None
                held = {}
                for jt in range(NJT):
                    if jt == NJT - 1:
                        nxt = (ic + 1, 0) if ic < 3 else None
                    else:
                        nxt = (ic, jt + 1)
                    if ic == 3 and jt == NJT - 1:
                        s_fin = s_cur          # jt16 score tiles, dead after
                    p_tiles = emit_exps(ic, jt, s_cur)
                    s_cur = emit_scores(*nxt) if nxt is not None else None
                    if jt < 8:
                        held[jt] = p_tiles
                        if jt == 5:
                            pvl = alloc_pvl(ic)
                        for fjt in flush_plan.get(jt, ()):
                            emit_pvl(ic, fjt, held.pop(fjt), pvl)
                    else:
                        emit_pvl(ic, jt, p_tiles, pvl)
                    if jt < 4:
                        # one i-row-block per jt: keeps the psum-tag ring and
                        # the DVE copy chain from parking >4 PE instructions
                        # (which would stall the whole in-order PE stream)
                        emit_outproj_it(ic - 1, osb, yps, jt)
                    if jt == 4 and ic < 3:
                        # sc(ic+1) q-features, needed by ic(ic+1)'s scores
                        emit_aqk_pair(ic + 1, (0, 1), tag="srot1")
                pvl_prev = pvl
            osb = emit_norm(3, pvl_prev, final=True)
            yps = [pvl_prev[0], pvl_prev[1], pvl_prev[2], pvl_prev[3],
                   s_fin[0][:, 0:512], s_fin[0][:, 512:1024],
                   s_fin[1][:, 0:512], s_fin[1][:, 512:1024]]
            for it in range(4):
                emit_outproj_it(3, osb, yps, it, final=True)

    nc.compile()
    return nc


def _prep_inputs(x, w_qkv, w_out, b_out, void_q, void_k, void_v,
                 attention_trace, temperature_factor):
    """Host-side sharding / layout prep. Returns in_maps for 8 cores."""
    temp = np.maximum(1.0 + np.abs(attention_trace) * temperature_factor,
                      1.0).reshape(HEADS).astype(np.float32)
    scale = (DIM ** -0.5) / temp                       # [16] per head
    # split the score scale between q (fixed 1/4) and k (4*scale_h) so both
    # fp8 operands stay in e4m3's normal range
    kcol_scale = np.repeat(4.0 * scale, D)             # [1024]
    wq_scaled = (w_qkv[:, 0:DIM] * 0.25).astype(np.float32)
    wk_scaled = (w_qkv[:, DIM:2 * DIM] * kcol_scale[None, :]).astype(np.float32)
    wv_full = w_qkv[:, 2 * DIM:3 * DIM]
    vk = (void_k.reshape(HEADS, D) * (4.0 * scale)[:, None]).astype(np.float32)
    vv = void_v.reshape(HEADS, D)

    ebias = np.zeros((P, 1), np.float32)
    ebias[1:, 0] = -100.0

    in_maps = []
    for core in range(8):
        b, hg = divmod(core, 4)
        h0 = hg * HPC
        cs = slice(h0 * D, (h0 + HPC) * D)             # 256 feature cols
        in_maps.append({
            "xT": np.ascontiguousarray(x[b].T),
            "wqkv": np.ascontiguousarray(
                np.concatenate([wq_scaled[:, cs], wk_scaled[:, cs],
                                wv_full[:, cs]], axis=1)),
            "wout": np.ascontiguousarray(w_out[cs, :]),
            # voidk rows: partition p = hh*64 + d, col = pair
            "voidk": np.ascontiguousarray(
                vk[h0:h0 + HPC].reshape(2, 2, D)       # [pair, hh, d]
                .transpose(1, 2, 0).reshape(P, 2)),
            "voidv": np.ascontiguousarray(vv[h0:h0 + HPC].reshape(1, 256)),
            "ebias_in": ebias,
        })
    return in_maps


def _run(in_maps, trace=False):
    from concourse import bass_utils
    if "nc" not in _cache:
        _cache["nc"] = _build()
    return bass_utils.run_bass_kernel_spmd(
        _cache["nc"], in_maps, core_ids=list(range(8)), trace=trace)


def kernel(x, w_qkv, w_out, b_out, void_q, void_k, void_v,
           attention_trace, temperature_factor):
    args = [np.asarray(a, dtype=np.float32) for a in
            (x, w_qkv, w_out, b_out, void_q, void_k, void_v,
             attention_trace, temperature_factor)]
    in_maps = _prep_inputs(*args)
    res = _run(in_maps)
    out = np.zeros((B, N, DIM), np.float32)
    for core in range(8):
        b = core // 4
        out[b] += res.results[core]["y"]
    out += args[3][None, None, :]                      # b_out
    return out


# revision 52
# speedup vs baseline: 1.0004x; 1.0004x over previous
"""BlanchotianAttention TRN2 kernel: 8 NeuronCores, data-parallel over batch (2)
x tensor-parallel over heads (4 heads/core).

v3 over the 260us baseline:
  - scores matmuls in fp8e4 DoubleRow perf mode (0.5 cyc/row): Q and K are
    quantized to fp8 during the stage-A PSUM evacuation copies. The DoubleRow
    second k-tile slot is zero-filled (contraction depth is only d=64). The
    dim^-0.5/temp score scale is split between q (x0.25) and k (x 4*scale_h)
    so both fp8 operands sit in e4m3's normal range.
  - inputs DMA directly into per-chunk float32r SBUF tiles (f32r is
    bit-identical to f32; rounding happens inside the PE). Per-chunk tiles
    matter: the Tile framework tracks dependencies per tile, so one big tile
    would make the first matmul wait for every input DMA.
  - DMA arrival order is tuned for the serial DMA pipe: interleaved
    (w_qk[ko], xT[ko, sc0]) pairs feed the first stage-A accumulation at pipe
    speed; later chunks stream behind. Stage A runs ko-major so accumulation
    tracks DMA arrival.
  - first exp fires at ~15us (vs ~44us) and ACT (the bottleneck engine at
    ~142us busy) runs with few gaps; tail norm/outproj is split across
    DVE+GPSIMD.

Layout (per core, batch b, head-group hg = heads h0..h0+3):
  - stage A-qk: psum = w.T @ xT -> QT/KT [d, seq] head-pair tiles, evacuated
    to fp8 qt8/kt8 (partitions 0:64 even head of pair, 64:128 odd head).
  - stage A-v: V = x @ w_v in [seq, d] layout, augmented per head with a ones
    block: V_aug[j, h*128 : h*128+128] = [v_h (64) | ones (64)] (f32r).
  - stage B/C per (512-wide i-chunk, head pair, 128-wide j-tile):
    S^T = K @ Q^T via one DoubleRow matmul per head,
    P = exp(S^T) on ACT (one [128,1024] activation covers both heads),
    PV+l fused f32r: matmul(lhsT=[v_h | ones], rhs=P) accumulates attn@v in
    PSUM rows 0-63 and the softmax denominator (broadcast) in rows 64-127.
  - void token: the void QUERY row is dropped by the reference so it is never
    computed. The void KEY/VALUE occupy j=2048 inside j-tile 16, zero-padded
    to 128 rows; a per-partition exp bias of -100 on that tile zeroes the pad
    rows' contributions.
  - normalize: reciprocal of the l rows + cross-base multiply -> O^T pair tile.
  - stage D: y_partial = O_norm @ w_out_shard; host sums partials over the 4
    head-group cores of each batch (+ b_out).
"""
import sys

sys.path.insert(0, "/opt/trn_rl_repo")

import numpy as np

DIM, HEADS, B, N = 1024, 16, 2, 2048
D = DIM // HEADS          # 64
HPC = HEADS // 4          # heads per core = 4
NJT = 17                  # j tiles (16 full + void/pad tile)
P = 128

_cache = {}
DEBUG = False


def _build():
    import concourse.bass as bass
    import concourse.mybir as mybir
    import concourse.tile as tile
    from concourse import bacc

    F32 = mybir.dt.float32
    F32R = mybir.dt.float32r
    BF16 = mybir.dt.bfloat16
    FP8 = mybir.dt.float8e4
    DR = mybir.MatmulPerfMode.DoubleRow
    Exp = mybir.ActivationFunctionType.Exp
    Rcp = mybir.ActivationFunctionType.Reciprocal

    nc = bacc.Bacc("TRN2", target_bir_lowering=False, debug=False)
    xT = nc.dram_tensor("xT", [DIM, N], F32R, kind="ExternalInput").ap()
    wqkv = nc.dram_tensor("wqkv", [DIM, 768], F32R, kind="ExternalInput").ap()
    wout = nc.dram_tensor("wout", [256, DIM], F32R, kind="ExternalInput").ap()
    voidk = nc.dram_tensor("voidk", [P, 2], F32, kind="ExternalInput").ap()
    voidv = nc.dram_tensor("voidv", [1, 256], F32, kind="ExternalInput").ap()
    ebias_in = nc.dram_tensor("ebias_in", [P, 1], F32, kind="ExternalInput").ap()
    y = nc.dram_tensor("y", [N, DIM], F32, kind="ExternalOutput").ap()
    if DEBUG:
        dbg_qt = nc.dram_tensor("dbg_qt", [P, 2, 2, N], FP8,
                                kind="ExternalOutput").ap()
        dbg_kt = nc.dram_tensor("dbg_kt", [P, 2, 2, NJT * P], FP8,
                                kind="ExternalOutput").ap()
        dbg_va = nc.dram_tensor("dbg_va", [P, NJT, 512], BF16,
                                kind="ExternalOutput").ap()
        dbg_pvl = nc.dram_tensor("dbg_pvl", [P, 512], F32,
                                 kind="ExternalOutput").ap()

    KO = DIM // P  # 8 k-tiles

    with tile.TileContext(nc) as tc:
        with tc.tile_pool(name="persist", bufs=1) as pp, \
             tc.tile_pool(name="work", bufs=1) as wp, \
             tc.tile_pool(name="psum", bufs=1, space="PSUM") as ps, \
             tc.tile_pool(name="loadA", bufs=2) as lp:

            # ---- constants ----
            ones = pp.tile([P, D], F32)
            nc.vector.memset(ones[:], 1.0)
            ebias = pp.tile([P, 1], F32)

            # ---- persistent SBUF tensors ----
            # qt8/kt8: [part, pair, doublerow-ktile, seq]; partitions 0:64 =
            # even head of the pair, 64:128 = odd head; ktile slot 1 is zeroed.
            qt8 = pp.tile([P, 2, 2, N], FP8)
            kt8 = pp.tile([P, 2, 2, NJT * P], FP8)
            va = pp.tile([P, NJT, 512], BF16)          # V_aug per j-tile
            wq_t = [pp.tile([P, 512], F32R, name=f"wq_{ko}") for ko in range(KO)]
            wv_t = [pp.tile([P, 256], F32R, name=f"wv_{ko}") for ko in range(KO)]
            wout_t = [pp.tile([P, DIM], F32R, name=f"wout_{h}") for h in range(2)]
            xt_t = [[pp.tile([P, 512], F32R, name=f"xt_{ko}_{sc}")
                     for sc in range(4)] for ko in range(KO)]

            # DoubleRow slot-1 zeros: off the DMA critical path
            nc.vector.memset(qt8[:, :, 1, :], 0.0)
            nc.gpsimd.memset(kt8[:, :, 1, :], 0.0)

            # V_aug ones blocks (DVE, depends only on `ones`)
            for jt in range(NJT):
                nc.vector.tensor_copy(
                    va[:, jt, :].rearrange("p (h c) -> p h c", c=P)[:, :, D:P],
                    ones[:, None, :].to_broadcast([P, 4, D]))

            # ---- input DMA: serial DMA pipe => arrival order is everything.
            # Nothing on the scalar (ACT) queue: DMA issue instructions with
            # ring-full waits clog ACT.SEQ and stall the exps behind them.
            for ko in range(KO):
                nc.sync.dma_start(wq_t[ko][:], wqkv[ko * P:(ko + 1) * P, 0:512])
                nc.sync.dma_start(xt_t[ko][0][:],
                                  xT[ko * P:(ko + 1) * P, 0:512])
            for ko in range(KO):
                nc.sync.dma_start(xt_t[ko][1][:],
                                  xT[ko * P:(ko + 1) * P, 512:1024])
            for ko in range(KO):
                nc.sync.dma_start(wv_t[ko][:],
                                  wqkv[ko * P:(ko + 1) * P, 512:768])
            for ko in range(KO):
                nc.sync.dma_start(xt_t[ko][2][:],
                                  xT[ko * P:(ko + 1) * P, 1024:1536])
            nc.sync.dma_start(ebias[:], ebias_in)
            vkt = lp.tile([P, 2], F32, tag="stg", name="vkt")
            nc.sync.dma_start(vkt[:], voidk)
            vvt = lp.tile([1, 256], F32, tag="stg", name="vvt")
            nc.sync.dma_start(vvt[:], voidv)
            for ko in range(KO):
                nc.sync.dma_start(xt_t[ko][3][:],
                                  xT[ko * P:(ko + 1) * P, 1536:2048])
            for h in range(2):
                nc.sync.dma_start(wout_t[h][:], wout[h * P:(h + 1) * P, :])

            # ---- stage A emit helpers ----
            def emit_aqk_pair(sc, fts, tag="srot0"):
                # fts: (0,1) or (2,3), both packed in ONE psum tile (the srot
                # tag ring is the scarce resource); ko-major so PSUM
                # accumulation tracks DMA arrival.
                acc = ps.tile([P, 1024], F32, tag=tag,
                              name=f"aqk_{sc}_{fts[0]}")
                for ko in range(KO):
                    for i, ft in enumerate(fts):
                        nc.tensor.matmul(
                            acc[:, i * 512:(i + 1) * 512],
                            wq_t[ko][:, ft * P:(ft + 1) * P],
                            xt_t[ko][sc][:],
                            start=(ko == 0), stop=(ko == KO - 1),
                        )
                for i, ft in enumerate(fts):
                    dst = qt8 if ft < 2 else kt8
                    nc.vector.tensor_copy(
                        dst[:, ft % 2, 0, sc * 512:(sc + 1) * 512],
                        acc[:, i * 512:(i + 1) * 512])

            def emit_av2(q, half, tag):
                # 2 avs per psum tile, one per 2KB bank: PSUM start=True
                # resets the whole bank, so two accumulation groups must not
                # share one
                acc = ps.tile([P, 1024], F32, tag=tag, name=f"avw_{q}_{half}")
                offs = (0, 512)
                stis = (2 * half, 2 * half + 1)
                for ko in range(KO):
                    for i, sti in enumerate(stis):
                        nc.tensor.matmul(
                            acc[:, offs[i]:offs[i] + 256],
                            xt_t[ko][q][:, sti * P:(sti + 1) * P],
                            wv_t[ko][:],
                            start=(ko == 0), stop=(ko == KO - 1),
                        )
                for i, sti in enumerate(stis):
                    st = 4 * q + sti
                    nc.vector.tensor_copy(
                        va[:, st, :].rearrange("p (h c) -> p h c", c=P)[:, :, 0:D],
                        acc[:, offs[i]:offs[i] + 256]
                        .rearrange("p (h c) -> p h c", c=D))

            def emit_void_setup():
                # void k column + pad zeros + V_aug void row
                nc.vector.tensor_copy(kt8[:, :, 0, 2048:2049], vkt[:])
                nc.vector.memset(kt8[:, :, 0, 2049:NJT * P], 0.0)
                va16 = va[:, 16, :]
                nc.vector.memset(
                    va16.rearrange("p (h c) -> p h c", c=P)[:, :, 0:D]
                    .bitcast(F32), 0.0)
                nc.vector.tensor_copy(
                    va16.rearrange("p (h c) -> p h c", c=P)[0:1, :, 0:D],
                    vvt[:].rearrange("p (h c) -> p h c", c=D))

            # ---- stage B/C/D emit helpers ----
            def emit_scores_pair(ic, jt, pair):
                isl = slice(ic * 512, (ic + 1) * 512)
                jsl = slice(jt * P, (jt + 1) * P)
                s_pair = ps.tile([P, 1024], F32, tag=f"srot{pair}",
                                 name=f"s_{ic}_{jt}_{pair}")
                for hh in range(2):
                    nc.tensor.matmul(
                        s_pair[:, hh * 512:(hh + 1) * 512],
                        kt8[hh * D:(hh + 1) * D, pair, :, jsl],
                        qt8[hh * D:(hh + 1) * D, pair, :, isl],
                        start=True, stop=True, perf_mode=DR)
                return s_pair

            def emit_scores(ic, jt):
                return [emit_scores_pair(ic, jt, pair) for pair in range(2)]

            def emit_exps(ic, jt, s_cur):
                p_tiles = []
                for pair in range(2):
                    p_pair = wp.tile([P, 1024], BF16, tag=f"pexp{pair}",
                                     bufs=6,
                                     name=f"p_{ic}_{jt}_{pair}")
                    if jt == 16:
                        nc.scalar.activation(p_pair[:], s_cur[pair][:], Exp,
                                             bias=ebias[:])
                    else:
                        nc.scalar.activation(p_pair[:], s_cur[pair][:], Exp)
                    p_tiles.append(p_pair)
                return p_tiles

            def emit_pvl(ic, jt, p_tiles, pvl):
                for pair in range(2):
                    for hh in range(2):
                        h = 2 * pair + hh
                        nc.tensor.matmul(
                            pvl[h][:],
                            va[:, jt, h * P:(h + 1) * P],
                            p_tiles[pair][:, hh * 512:(hh + 1) * 512],
                            start=(jt == 0), stop=(jt == 16),
                        )

            def emit_exp_pvl(ic, jt, s_cur, pvl, nxt, mid=None):
                """exp(jt) ; scores(nxt) ; [mid()] ; pvl(jt)."""
                p_tiles = emit_exps(ic, jt, s_cur)
                s_nxt = emit_scores(*nxt) if nxt is not None else None
                if mid is not None:
                    mid()
                emit_pvl(ic, jt, p_tiles, pvl)
                return s_nxt

            def emit_norm(ic, pvl, final=False):
                """normalize pvl -> osb SBUF tiles."""
                osb = [wp.tile([P, 512], F32R,
                               tag=f"osbf{pair}" if final else f"osb{pair}",
                               bufs=2, name=f"osb{pair}_{ic}")
                       for pair in range(2)]
                rsbs = []
                for h in range(4):
                    r_sb = lp.tile([P, 512], F32, tag="rsbf" if final else "rsb",
                                   bufs=4 if final else 2,
                                   name=f"rsb_{ic}_{h}")
                    rsbs.append(r_sb)
                    nc.vector.reciprocal(r_sb[D:P, :], pvl[h][D:P, :])
                    if not final:
                        pair, hh = divmod(h, 2)
                        nc.vector.tensor_tensor(
                            osb[pair][hh * D:(hh + 1) * D, :],
                            pvl[h][0:D, :], r_sb[D:P, :],
                            mybir.AluOpType.mult)
                if final:
                    # all recips first, then the mults
                    for h in range(4):
                        pair, hh = divmod(h, 2)
                        nc.vector.tensor_tensor(
                            osb[pair][hh * D:(hh + 1) * D, :],
                            pvl[h][0:D, :], rsbs[h][D:P, :],
                            mybir.AluOpType.mult)
                return osb

            def emit_outproj_it(ic, osb, yps, it, final=False):
                # yps are DEAD psum tiles (the drained pvl accumulators / last
                # score tiles) written in place: allocating fresh psum tiles
                # would share slots with the next ic's pvl accumulators via
                # the pool's LIFO allocator and serialize the whole tail
                ysb = wp.tile([P, DIM], F32, tag="ysbf" if final else "ysb",
                              bufs=2 if final else 4,
                              name=f"ysb_{ic}_{it}")
                for oc in range(2):
                    yp = yps[it * 2 + oc]
                    for pair in range(2):
                        nc.tensor.matmul(
                            yp[:],
                            osb[pair][:, it * P:(it + 1) * P],
                            wout_t[pair][:, oc * 512:(oc + 1) * 512],
                            start=(pair == 0), stop=(pair == 1),
                        )
                    if final and oc == 1:
                        # ACT is idle after the last exp and can read PSUM
                        nc.scalar.copy(ysb[:, oc * 512:(oc + 1) * 512], yp[:])
                    else:
                        nc.vector.tensor_copy(ysb[:, oc * 512:(oc + 1) * 512],
                                              yp[:])
                    if final:
                        # don't wait for the other half: stream each 512-col
                        # half as soon as its copy lands
                        nc.sync.dma_start(
                            y[ic * 512 + it * P: ic * 512 + (it + 1) * P,
                              oc * 512:(oc + 1) * 512],
                            ysb[:, oc * 512:(oc + 1) * 512])
                if not final:
                    nc.sync.dma_start(
                        y[ic * 512 + it * P: ic * 512 + (it + 1) * P, :],
                        ysb[:])

            def alloc_pvl(ic):
                return [ps.tile([P, 512], F32, tag=f"pvl{h}", name=f"pvl{h}_{ic}")
                        for h in range(4)]

            # ---- main schedule ----
            # sc0's 4 q/k features run upfront, ko-major (tracking DMA
            # arrival), packed into the two srot tiles' halves so both score
            # pairs can fire back-to-back; ic1..3 plain.
            pvl = alloc_pvl(0)
            accp = [ps.tile([P, 1024], F32, tag=f"srot{i}", name=f"a0_p{i}")
                    for i in range(2)]
            for ko in range(KO):
                # (ft, pair-acc, col offset): pair0 = ft0|ft2, pair1 = ft1|ft3
                for ft, pair in ((0, 0), (2, 0), (1, 1), (3, 1)):
                    off = 0 if ft < 2 else 512
                    nc.tensor.matmul(
                        accp[pair][:, off:off + 512],
                        wq_t[ko][:, ft * P:(ft + 1) * P],
                        xt_t[ko][0][:],
                        start=(ko == 0), stop=(ko == KO - 1),
                    )
            for pair in range(2):
                nc.vector.tensor_copy(qt8[:, pair, 0, 0:512],
                                      accp[pair][:, 0:512])
                nc.vector.tensor_copy(kt8[:, pair, 0, 0:512],
                                      accp[pair][:, 512:1024])
            s00_p0 = emit_scores_pair(0, 0, 0)
            s00_p1 = emit_scores_pair(0, 0, 1)
            s_cur = [s00_p0, s00_p1]
            # jts 0-2: exps + next-scores only; their pvl matmuls are emitted
            # after the avs so the av matmuls (gated on the w_v DMA) never sit
            # between scores in the in-order PE stream
            deferred = []
            for jt in range(0, 3):
                p_tiles = emit_exps(0, jt, s_cur)
                if jt == 1:
                    # av accs take ring slots here so they only wait early
                    # exp reads, keeping pvl(0,*) fed without stalling the
                    # jt3+ score chain
                    emit_av2(0, 0, "srot1")
                if jt == 2:
                    emit_av2(0, 1, "srot1")
                s_cur = emit_scores(0, jt + 1)
                deferred.append((jt, p_tiles))
            for jt, p_tiles in deferred:
                emit_pvl(0, jt, p_tiles, pvl)
            for q in range(1, 4):
                emit_aqk_pair(q, (2, 3), tag="srot0")
                emit_av2(q, 0, "srot1")
                if q == 2:
                    emit_void_setup()
                lo, hi = 4 * q - 1, 4 * q + 3   # jts whose next-scores live in sc q
                for jt in range(lo, hi if q < 3 else NJT):
                    nxt = (0, jt + 1) if jt < NJT - 1 else (1, 0)
                    s_cur = emit_exp_pvl(0, jt, s_cur, pvl, nxt)
                    if jt == lo + 1:
                        emit_av2(q, 1, "srot0")
                    if jt == lo + 2 and q == 2:
                        # sc1 q-features feed ic1's scores (at ic0-jt16);
                        # emitted in window q=2 where the srot1 ring is calmer
                        emit_aqk_pair(1, (0, 1), tag="srot1")

            if DEBUG:
                nc.sync.dma_start(dbg_qt, qt8[:])
                nc.sync.dma_start(dbg_kt, kt8[:])
                nc.sync.dma_start(dbg_va, va[:])
                dbg_sb = wp.tile([P, DIM], F32, tag="ysb", bufs=4,
                                 name="dbg_sb")
                nc.vector.tensor_copy(dbg_sb[:, 0:512], pvl[0][:])
                nc.sync.dma_start(dbg_pvl, dbg_sb[:, 0:512])
            pvl_prev = pvl
            s_fin = None
            # outproj(ic-1) writes its y psum in place into the drained
            # pvl(ic-1) tiles; pvl(ic)'s tag slots alias those banks, so its
            # allocation + first jts' matmuls are deferred past the outproj
            # writes (p tiles wait in the pexp ring)
            flush_plan = {5: (0, 1, 2), 6: (3, 4, 5), 7: (6, 7)}
            for ic in range(1, 4):
                osb = emit_norm(ic - 1, pvl_prev)
                yps = [pvl_prev[k % 4] for k in range(8)]
                pvl = None
                held = {}
                for jt in range(NJT):
                    if jt == NJT - 1:
                        nxt = (ic + 1, 0) if ic < 3 else None
                    else:
                        nxt = (ic, jt + 1)
                    if ic == 3 and jt == NJT - 1:
                        s_fin = s_cur          # jt16 score tiles, dead after
                    p_tiles = emit_exps(ic, jt, s_cur)
                    s_cur = emit_scores(*nxt) if nxt is not None else None
                    if jt < 8:
                        held[jt] = p_tiles
                        if jt == 5:
                            pvl = alloc_pvl(ic)
                        for fjt in flush_plan.get(jt, ()):
                            emit_pvl(ic, fjt, held.pop(fjt), pvl)
                    else:
                        emit_pvl(ic, jt, p_tiles, pvl)
                    if jt < 4:
                        # one i-row-block per jt: keeps the psum-tag ring and
                        # the DVE copy chain from parking >4 PE instructions
                        # (which would stall the whole in-order PE stream)
                        emit_outproj_it(ic - 1, osb, yps, jt)
                    if jt == 4 and ic < 3:
                        # sc(ic+1) q-features, needed by ic(ic+1)'s scores
                        emit_aqk_pair(ic + 1, (0, 1), tag="srot1")
                pvl_prev = pvl
            osb = emit_norm(3, pvl_prev, final=True)
            yps = [pvl_prev[0], pvl_prev[1], pvl_prev[2], pvl_prev[3],
                   s_fin[0][:, 0:512], s_fin[0][:, 512:1024],
                   s_fin[1][:, 0:512], s_fin[1][:, 512:1024]]
            for it in range(4):
                emit_outproj_it(3, osb, yps, it, final=True)

    nc.compile()
    return nc


def _prep_inputs(x, w_qkv, w_out, b_out, void_q, void_k, void_v,
                 attention_trace, temperature_factor):
    """Host-side sharding / layout prep. Returns in_maps for 8 cores."""
    temp = np.maximum(1.0 + np.abs(attention_trace) * temperature_factor,
                      1.0).reshape(HEADS).astype(np.float32)
    scale = (DIM ** -0.5) / temp                       # [16] per head
    # split the score scale between q (fixed 1/4) and k (4*scale_h) so both
    # fp8 operands stay in e4m3's normal range
    kcol_scale = np.repeat(4.0 * scale, D)             # [1024]
    wq_scaled = (w_qkv[:, 0:DIM] * 0.25).astype(np.float32)
    wk_scaled = (w_qkv[:, DIM:2 * DIM] * kcol_scale[None, :]).astype(np.float32)
    wv_full = w_qkv[:, 2 * DIM:3 * DIM]
    vk = (void_k.reshape(HEADS, D) * (4.0 * scale)[:, None]).astype(np.float32)
    vv = void_v.reshape(HEADS, D)

    ebias = np.zeros((P, 1), np.float32)
    ebias[1:, 0] = -100.0

    in_maps = []
    for core in range(8):
        b, hg = divmod(core, 4)
        h0 = hg * HPC
        cs = slice(h0 * D, (h0 + HPC) * D)             # 256 feature cols
        in_maps.append({
            "xT": np.ascontiguousarray(x[b].T),
            "wqkv": np.ascontiguousarray(
                np.concatenate([wq_scaled[:, cs], wk_scaled[:, cs],
                                wv_full[:, cs]], axis=1)),
            "wout": np.ascontiguousarray(w_out[cs, :]),
            # voidk rows: partition p = hh*64 + d, col = pair
            "voidk": np.ascontiguousarray(
                vk[h0:h0 + HPC].reshape(2, 2, D)       # [pair, hh, d]
                .transpose(1, 2, 0).reshape(P, 2)),
            "voidv": np.ascontiguousarray(vv[h0:h0 + HPC].reshape(1, 256)),
            "ebias_in": ebias,
        })
    return in_maps


def _run(in_maps, trace=False):
    from concourse import bass_utils
    if "nc" not in _cache:
        _cache["nc"] = _build()
    return bass_utils.run_bass_kernel_spmd(
        _cache["nc"], in_maps, core_ids=list(range(8)), trace=trace)


def kernel(x, w_qkv, w_out, b_out, void_q, void_k, void_v,
           attention_trace, temperature_factor):
    args = [np.asarray(a, dtype=np.float32) for a in
            (x, w_qkv, w_out, b_out, void_q, void_k, void_v,
             attention_trace, temperature_factor)]
    in_maps = _prep_inputs(*args)
    res = _run(in_maps)
    out = np.zeros((B, N, DIM), np.float32)
    for core in range(8):
        b = core // 4
        out[b] += res.results[core]["y"]
    out += args[3][None, None, :]                      # b_out
    return out


# revision 58
# speedup vs baseline: 1.0209x; 1.0205x over previous
"""BlanchotianAttention TRN2 kernel: 8 NeuronCores, data-parallel over batch (2)
x tensor-parallel over heads (4 heads/core). 210.9us cost-model exec
(260.1us baseline), rel err 1.22e-2 (gate 2e-2).

Key techniques over the baseline:
  - scores matmuls in fp8e4 DoubleRow perf mode (0.5 cyc/row): Q and K are
    quantized to fp8 during the stage-A PSUM evacuation copies. The DoubleRow
    second k-tile slot is zero-filled (contraction depth is only d=64). The
    dim^-0.5/temp score scale is split between q (x0.25) and k (x 4*scale_h)
    so both fp8 operands sit in e4m3's normal range.
  - exp output P and V_aug are bf16: same matmul rate as f32r, half the SBUF,
    ~2e-3 error contribution.
  - inputs DMA directly into per-chunk float32r SBUF tiles (f32r is
    bit-identical to f32; rounding happens inside the PE). Per-chunk tiles
    matter: the Tile framework tracks dependencies per tile, so one big tile
    would make the first matmul wait for every input DMA.
  - DMA arrival order is tuned for the serial DMA pipe: interleaved
    (w_qk[ko], xT[ko, sc0]) pairs feed the first stage-A accumulation at pipe
    speed; later chunks stream behind, ko-major stage A tracks arrival. No
    input DMAs on the scalar queue: their ring-full waits clog ACT.SEQ (4-deep
    wait queue) and stall the exps behind them. First exp at ~17.5us vs ~44us.
  - the two srot PSUM tags are the scarce resource (scores need one rotation
    per j-tile per pair); window stage-A accumulators pack 2 features or
    2 v-blocks per [128,1024] tile to minimize ring insertions. PSUM
    start=True resets a whole 2KB bank, so no two accumulation groups may
    share one (av accumulators use one group per bank).
  - outproj writes its y psum in place into the drained pvl accumulators of
    the previous i-chunk; fresh psum tiles would entangle slot rings via the
    pool's LIFO allocator and serialize the tail. The next chunk's pvl
    allocation is deferred to jt5 (p tiles wait in the 6-deep pexp ring) so
    the banks hand over cleanly. One outproj row-block per j-tile keeps
    blocked PE instructions under the 4-deep park limit, past which the
    whole in-order PE stream stalls.
  - final outproj reuses the dead jt16 score tiles as y psum, streams each
    512-col half to DRAM as its copy lands, and puts half the copies on ACT
    (idle after the last exp; can read PSUM, which GPSIMD cannot).

Layout (per core, batch b, head-group hg = heads h0..h0+3):
  - stage A-qk: psum = w.T @ xT -> QT/KT [d, seq] head-pair tiles, evacuated
    to fp8 qt8/kt8 (partitions 0:64 even head of pair, 64:128 odd head).
  - stage A-v: V = x @ w_v in [seq, d] layout, augmented per head with a ones
    block: V_aug[j, h*128 : h*128+128] = [v_h (64) | ones (64)] (f32r).
  - stage B/C per (512-wide i-chunk, head pair, 128-wide j-tile):
    S^T = K @ Q^T via one DoubleRow matmul per head,
    P = exp(S^T) on ACT (one [128,1024] activation covers both heads),
    PV+l fused f32r: matmul(lhsT=[v_h | ones], rhs=P) accumulates attn@v in
    PSUM rows 0-63 and the softmax denominator (broadcast) in rows 64-127.
  - void token: the void QUERY row is dropped by the reference so it is never
    computed. The void KEY/VALUE occupy j=2048 inside j-tile 16, zero-padded
    to 128 rows; a per-partition exp bias of -100 on that tile zeroes the pad
    rows' contributions.
  - normalize: reciprocal of the l rows + cross-base multiply -> O^T pair tile.
  - stage D: y_partial = O_norm @ w_out_shard; host sums partials over the 4
    head-group cores of each batch (+ b_out).
"""
import sys

sys.path.insert(0, "/opt/trn_rl_repo")

import numpy as np

DIM, HEADS, B, N = 1024, 16, 2, 2048
D = DIM // HEADS          # 64
HPC = HEADS // 4          # heads per core = 4
NJT = 17                  # j tiles (16 full + void/pad tile)
P = 128

_cache = {}
DEBUG = False


def _build():
    import concourse.bass as bass
    import concourse.mybir as mybir
    import concourse.tile as tile
    from concourse import bacc

    F32 = mybir.dt.float32
    F32R = mybir.dt.float32r
    BF16 = mybir.dt.bfloat16
    FP8 = mybir.dt.float8e4
    DR = mybir.MatmulPerfMode.DoubleRow
    Exp = mybir.ActivationFunctionType.Exp
    Rcp = mybir.ActivationFunctionType.Reciprocal

    nc = bacc.Bacc("TRN2", target_bir_lowering=False, debug=False)
    xT = nc.dram_tensor("xT", [DIM, N], F32R, kind="ExternalInput").ap()
    wqkv = nc.dram_tensor("wqkv", [DIM, 768], F32R, kind="ExternalInput").ap()
    wout = nc.dram_tensor("wout", [256, DIM], F32R, kind="ExternalInput").ap()
    voidk = nc.dram_tensor("voidk", [P, 2], F32, kind="ExternalInput").ap()
    voidv = nc.dram_tensor("voidv", [1, 256], F32, kind="ExternalInput").ap()
    ebias_in = nc.dram_tensor("ebias_in", [P, 1], F32, kind="ExternalInput").ap()
    y = nc.dram_tensor("y", [N, DIM], F32, kind="ExternalOutput").ap()
    if DEBUG:
        dbg_qt = nc.dram_tensor("dbg_qt", [P, 2, 2, N], FP8,
                                kind="ExternalOutput").ap()
        dbg_kt = nc.dram_tensor("dbg_kt", [P, 2, 2, NJT * P], FP8,
                                kind="ExternalOutput").ap()
        dbg_va = nc.dram_tensor("dbg_va", [P, NJT, 512], BF16,
                                kind="ExternalOutput").ap()
        dbg_pvl = nc.dram_tensor("dbg_pvl", [P, 512], F32,
                                 kind="ExternalOutput").ap()

    KO = DIM // P  # 8 k-tiles

    with tile.TileContext(nc) as tc:
        with tc.tile_pool(name="persist", bufs=1) as pp, \
             tc.tile_pool(name="work", bufs=1) as wp, \
             tc.tile_pool(name="psum", bufs=1, space="PSUM") as ps, \
             tc.tile_pool(name="loadA", bufs=2) as lp:

            # ---- constants ----
            ones = pp.tile([P, D], F32)
            nc.vector.memset(ones[:], 1.0)
            ebias = pp.tile([P, 1], F32)

            # ---- persistent SBUF tensors ----
            # qt8/kt8: [part, pair, doublerow-ktile, seq]; partitions 0:64 =
            # even head of the pair, 64:128 = odd head; ktile slot 1 is zeroed.
            qt8 = pp.tile([P, 2, 2, N], FP8)
            kt8 = pp.tile([P, 2, 2, NJT * P], FP8)
            va = pp.tile([P, NJT, 512], BF16)          # V_aug per j-tile
            wq_t = [pp.tile([P, 512], F32R, name=f"wq_{ko}") for ko in range(KO)]
            wv_t = [pp.tile([P, 256], F32R, name=f"wv_{ko}") for ko in range(KO)]
            wout_t = [pp.tile([P, DIM], F32R, name=f"wout_{h}") for h in range(2)]
            xt_t = [[pp.tile([P, 512], F32R, name=f"xt_{ko}_{sc}")
                     for sc in range(4)] for ko in range(KO)]

            # DoubleRow slot-1 zeros: off the DMA critical path
            nc.vector.memset(qt8[:, :, 1, :], 0.0)
            nc.gpsimd.memset(kt8[:, :, 1, :], 0.0)

            # V_aug ones blocks (DVE, depends only on `ones`)
            for jt in range(NJT):
                nc.vector.tensor_copy(
                    va[:, jt, :].rearrange("p (h c) -> p h c", c=P)[:, :, D:P],
                    ones[:, None, :].to_broadcast([P, 4, D]))

            # ---- input DMA: serial DMA pipe => arrival order is everything.
            # Nothing on the scalar (ACT) queue: DMA issue instructions with
            # ring-full waits clog ACT.SEQ and stall the exps behind them.
            for ko in range(KO):
                nc.sync.dma_start(wq_t[ko][:], wqkv[ko * P:(ko + 1) * P, 0:512])
                nc.sync.dma_start(xt_t[ko][0][:],
                                  xT[ko * P:(ko + 1) * P, 0:512])
            for ko in range(KO):
                nc.sync.dma_start(xt_t[ko][1][:],
                                  xT[ko * P:(ko + 1) * P, 512:1024])
            for ko in range(KO):
                nc.sync.dma_start(wv_t[ko][:],
                                  wqkv[ko * P:(ko + 1) * P, 512:768])
            for ko in range(KO):
                nc.sync.dma_start(xt_t[ko][2][:],
                                  xT[ko * P:(ko + 1) * P, 1024:1536])
            nc.sync.dma_start(ebias[:], ebias_in)
            vkt = lp.tile([P, 2], F32, tag="stg", name="vkt")
            nc.sync.dma_start(vkt[:], voidk)
            vvt = lp.tile([1, 256], F32, tag="stg", name="vvt")
            nc.sync.dma_start(vvt[:], voidv)
            for ko in range(KO):
                nc.sync.dma_start(xt_t[ko][3][:],
                                  xT[ko * P:(ko + 1) * P, 1536:2048])
            for h in range(2):
                nc.sync.dma_start(wout_t[h][:], wout[h * P:(h + 1) * P, :])

            # ---- stage A emit helpers ----
            def emit_aqk_pair(sc, fts, tag="srot0"):
                # fts: (0,1) or (2,3). tag may be a single srot tag (both
                # features packed in ONE [128,1024] tile = one ring slot) or
                # a (tagA, tagB) pair of [128,512] single-bank tiles (used in
                # early ic0 when the pvl banks are still free). ko-major so
                # PSUM accumulation tracks DMA arrival.
                if isinstance(tag, tuple):
                    accs = [ps.tile([P, 512], F32, tag=t,
                                    name=f"aqk_{sc}_{fts[i]}")
                            for i, t in enumerate(tag)]
                    regions = [accs[0][:, 0:512], accs[1][:, 0:512]]
                else:
                    acc = ps.tile([P, 1024], F32, tag=tag,
                                  name=f"aqk_{sc}_{fts[0]}")
                    regions = [acc[:, 0:512], acc[:, 512:1024]]
                for ko in range(KO):
                    for i, ft in enumerate(fts):
                        nc.tensor.matmul(
                            regions[i],
                            wq_t[ko][:, ft * P:(ft + 1) * P],
                            xt_t[ko][sc][:],
                            start=(ko == 0), stop=(ko == KO - 1),
                        )
                for i, ft in enumerate(fts):
                    dst = qt8 if ft < 2 else kt8
                    nc.vector.tensor_copy(
                        dst[:, ft % 2, 0, sc * 512:(sc + 1) * 512],
                        regions[i])

            def emit_av2(q, half, tag):
                # 2 avs with one accumulation group per 2KB bank: PSUM
                # start=True resets the whole bank, so two groups must not
                # share one. tag: single srot tag ([128,1024], groups in its
                # two banks) or a (tagA, tagB) pair of single-bank tiles.
                stis = (2 * half, 2 * half + 1)
                if isinstance(tag, tuple):
                    accs = [ps.tile([P, 512], F32, tag=t,
                                    name=f"avw_{q}_{half}_{i}")
                            for i, t in enumerate(tag)]
                    regions = [accs[0][:, 0:256], accs[1][:, 0:256]]
                else:
                    acc = ps.tile([P, 1024], F32, tag=tag,
                                  name=f"avw_{q}_{half}")
                    regions = [acc[:, 0:256], acc[:, 512:768]]
                for ko in range(KO):
                    for i, sti in enumerate(stis):
                        nc.tensor.matmul(
                            regions[i],
                            xt_t[ko][q][:, sti * P:(sti + 1) * P],
                            wv_t[ko][:],
                            start=(ko == 0), stop=(ko == KO - 1),
                        )
                for i, sti in enumerate(stis):
                    st = 4 * q + sti
                    nc.vector.tensor_copy(
                        va[:, st, :].rearrange("p (h c) -> p h c", c=P)[:, :, 0:D],
                        regions[i].rearrange("p (h c) -> p h c", c=D))

            def emit_void_setup():
                # void k column + pad zeros + V_aug void row
                nc.vector.tensor_copy(kt8[:, :, 0, 2048:2049], vkt[:])
                nc.vector.memset(kt8[:, :, 0, 2049:NJT * P], 0.0)
                va16 = va[:, 16, :]
                nc.vector.memset(
                    va16.rearrange("p (h c) -> p h c", c=P)[:, :, 0:D]
                    .bitcast(F32), 0.0)
                nc.vector.tensor_copy(
                    va16.rearrange("p (h c) -> p h c", c=P)[0:1, :, 0:D],
                    vvt[:].rearrange("p (h c) -> p h c", c=D))

            # ---- stage B/C/D emit helpers ----
            def emit_scores_pair(ic, jt, pair):
                isl = slice(ic * 512, (ic + 1) * 512)
                jsl = slice(jt * P, (jt + 1) * P)
                s_pair = ps.tile([P, 1024], F32, tag=f"srot{pair}",
                                 name=f"s_{ic}_{jt}_{pair}")
                for hh in range(2):
                    nc.tensor.matmul(
                        s_pair[:, hh * 512:(hh + 1) * 512],
                        kt8[hh * D:(hh + 1) * D, pair, :, jsl],
                        qt8[hh * D:(hh + 1) * D, pair, :, isl],
                        start=True, stop=True, perf_mode=DR)
                return s_pair

            def emit_scores(ic, jt):
                return [emit_scores_pair(ic, jt, pair) for pair in range(2)]

            def emit_exps(ic, jt, s_cur):
                p_tiles = []
                for pair in range(2):
                    p_pair = wp.tile([P, 1024], BF16, tag=f"pexp{pair}",
                                     bufs=6,
                                     name=f"p_{ic}_{jt}_{pair}")
                    if jt == 16:
                        nc.scalar.activation(p_pair[:], s_cur[pair][:], Exp,
                                             bias=ebias[:])
                    else:
                        nc.scalar.activation(p_pair[:], s_cur[pair][:], Exp)
                    p_tiles.append(p_pair)
                return p_tiles

            def emit_pvl(ic, jt, p_tiles, pvl):
                for pair in range(2):
                    for hh in range(2):
                        h = 2 * pair + hh
                        nc.tensor.matmul(
                            pvl[h][:],
                            va[:, jt, h * P:(h + 1) * P],
                            p_tiles[pair][:, hh * 512:(hh + 1) * 512],
                            start=(jt == 0), stop=(jt == 16),
                        )

            def emit_exp_pvl(ic, jt, s_cur, pvl, nxt, mid=None):
                """exp(jt) ; scores(nxt) ; [mid()] ; pvl(jt)."""
                p_tiles = emit_exps(ic, jt, s_cur)
                s_nxt = emit_scores(*nxt) if nxt is not None else None
                if mid is not None:
                    mid()
                emit_pvl(ic, jt, p_tiles, pvl)
                return s_nxt

            def emit_norm(ic, pvl, final=False):
                """normalize pvl -> osb SBUF tiles."""
                osb = [wp.tile([P, 512], F32R,
                               tag=f"osbf{pair}" if final else f"osb{pair}",
                               bufs=2, name=f"osb{pair}_{ic}")
                       for pair in range(2)]
                rsbs = []
                for h in range(4):
                    r_sb = lp.tile([P, 512], F32, tag="rsbf" if final else "rsb",
                                   bufs=4 if final else 2,
                                   name=f"rsb_{ic}_{h}")
                    rsbs.append(r_sb)
                    nc.vector.reciprocal(r_sb[D:P, :], pvl[h][D:P, :])
                    if not final:
                        pair, hh = divmod(h, 2)
                        nc.vector.tensor_tensor(
                            osb[pair][hh * D:(hh + 1) * D, :],
                            pvl[h][0:D, :], r_sb[D:P, :],
                            mybir.AluOpType.mult)
                if final:
                    # all recips first, then the mults
                    for h in range(4):
                        pair, hh = divmod(h, 2)
                        nc.vector.tensor_tensor(
                            osb[pair][hh * D:(hh + 1) * D, :],
                            pvl[h][0:D, :], rsbs[h][D:P, :],
                            mybir.AluOpType.mult)
                return osb

            def emit_outproj_it(ic, osb, yps, it, final=False):
                # yps are DEAD psum tiles (the drained pvl accumulators / last
                # score tiles) written in place: allocating fresh psum tiles
                # would share slots with the next ic's pvl accumulators via
                # the pool's LIFO allocator and serialize the whole tail
                ysb = wp.tile([P, DIM], F32, tag="ysbf" if final else "ysb",
                              bufs=2 if final else 4,
                              name=f"ysb_{ic}_{it}")
                for oc in range(2):
                    yp = yps[it * 2 + oc]
                    for pair in range(2):
                        nc.tensor.matmul(
                            yp[:],
                            osb[pair][:, it * P:(it + 1) * P],
                            wout_t[pair][:, oc * 512:(oc + 1) * 512],
                            start=(pair == 0), stop=(pair == 1),
                        )
                    if final and oc == 1:
                        # ACT is idle after the last exp and can read PSUM
                        nc.scalar.copy(ysb[:, oc * 512:(oc + 1) * 512], yp[:])
                    else:
                        nc.vector.tensor_copy(ysb[:, oc * 512:(oc + 1) * 512],
                                              yp[:])
                    if final:
                        # don't wait for the other half: stream each 512-col
                        # half as soon as its copy lands
                        nc.sync.dma_start(
                            y[ic * 512 + it * P: ic * 512 + (it + 1) * P,
                              oc * 512:(oc + 1) * 512],
                            ysb[:, oc * 512:(oc + 1) * 512])
                if not final:
                    nc.sync.dma_start(
                        y[ic * 512 + it * P: ic * 512 + (it + 1) * P, :],
                        ysb[:])

            def alloc_pvl(ic):
                return [ps.tile([P, 512], F32, tag=f"pvl{h}", name=f"pvl{h}_{ic}")
                        for h in range(4)]

            # ---- main schedule ----
            # sc0's 4 q/k features run upfront, ko-major (tracking DMA
            # arrival), packed into the two srot tiles' halves so both score
            # pairs can fire back-to-back; ic1..3 plain.
            # pvl allocation is always deferred to jt5 so the previous
            # owner of the banks (outproj writes / early-window accs) is done
            flush_plan = {5: (0, 1, 2), 6: (3, 4, 5), 7: (6, 7)}
            accp = [ps.tile([P, 1024], F32, tag=f"srot{i}", name=f"a0_p{i}")
                    for i in range(2)]
            for ko in range(KO):
                # (ft, pair-acc, col offset): pair0 = ft0|ft2, pair1 = ft1|ft3
                for ft, pair in ((0, 0), (2, 0), (1, 1), (3, 1)):
                    off = 0 if ft < 2 else 512
                    nc.tensor.matmul(
                        accp[pair][:, off:off + 512],
                        wq_t[ko][:, ft * P:(ft + 1) * P],
                        xt_t[ko][0][:],
                        start=(ko == 0), stop=(ko == KO - 1),
                    )
            for pair in range(2):
                nc.vector.tensor_copy(qt8[:, pair, 0, 0:512],
                                      accp[pair][:, 0:512])
                nc.vector.tensor_copy(kt8[:, pair, 0, 0:512],
                                      accp[pair][:, 512:1024])
            s00_p0 = emit_scores_pair(0, 0, 0)
            s00_p1 = emit_scores_pair(0, 0, 1)
            s_cur = [s00_p0, s00_p1]
            # ic0 flat jt loop. pvl(0) allocation is deferred to jt5 (p tiles
            # wait in the pexp ring), so during jts 0-4 the four pvl banks are
            # free: the early window accumulators (avs for sc0/sc1-keys,
            # sc1 q-features) live there instead of stealing srot ring slots
            # from the score pipeline.
            hooks = {
                1: lambda: emit_av2(0, 0, ("pvl0", "pvl1")),
                2: lambda: emit_av2(0, 1, ("pvl2", "pvl3")),
                3: lambda: emit_aqk_pair(1, (2, 3), ("pvl0", "pvl1")),
                4: lambda: emit_aqk_pair(1, (0, 1), ("pvl2", "pvl3")),
                5: lambda: emit_av2(1, 0, "srot1"),
                6: lambda: emit_av2(1, 1, "srot0"),
                7: lambda: (emit_aqk_pair(2, (2, 3), "srot0"),
                            emit_av2(2, 0, "srot1")),
                8: lambda: (emit_av2(2, 1, "srot0"), emit_void_setup()),
                11: lambda: (emit_aqk_pair(3, (2, 3), "srot0"),
                             emit_av2(3, 0, "srot1")),
                12: lambda: emit_av2(3, 1, "srot0"),
            }
            held = {}
            pvl = None
            for jt in range(NJT):
                p_tiles = emit_exps(0, jt, s_cur)
                if jt in hooks:
                    hooks[jt]()
                nxt = (0, jt + 1) if jt < NJT - 1 else (1, 0)
                s_cur = emit_scores(*nxt)
                if jt < 8:
                    held[jt] = p_tiles
                    if jt == 5:
                        pvl = alloc_pvl(0)
                    for fjt in flush_plan.get(jt, ()):
                        emit_pvl(0, fjt, held.pop(fjt), pvl)
                else:
                    emit_pvl(0, jt, p_tiles, pvl)

            if DEBUG:
                nc.sync.dma_start(dbg_qt, qt8[:])
                nc.sync.dma_start(dbg_kt, kt8[:])
                nc.sync.dma_start(dbg_va, va[:])
                dbg_sb = wp.tile([P, DIM], F32, tag="ysb", bufs=4,
                                 name="dbg_sb")
                nc.vector.tensor_copy(dbg_sb[:, 0:512], pvl[0][:])
                nc.sync.dma_start(dbg_pvl, dbg_sb[:, 0:512])
            pvl_prev = pvl
            s_fin = None
            # outproj(ic-1) writes its y psum in place into the drained
            # pvl(ic-1) tiles; pvl(ic)'s tag slots alias those banks, so its
            # allocation + first jts' matmuls are deferred past the outproj
            # writes (p tiles wait in the pexp ring)
            for ic in range(1, 4):
                osb = emit_norm(ic - 1, pvl_prev)
                yps = [pvl_prev[k % 4] for k in range(8)]
                pvl = None
                held = {}
                for jt in range(NJT):
                    if jt == NJT - 1:
                        nxt = (ic + 1, 0) if ic < 3 else None
                    else:
                        nxt = (ic, jt + 1)
                    if ic == 3 and jt == NJT - 1:
                        s_fin = s_cur          # jt16 score tiles, dead after
                    p_tiles = emit_exps(ic, jt, s_cur)
                    s_cur = emit_scores(*nxt) if nxt is not None else None
                    if jt < 8:
                        held[jt] = p_tiles
                        if jt == 5:
                            pvl = alloc_pvl(ic)
                        for fjt in flush_plan.get(jt, ()):
                            emit_pvl(ic, fjt, held.pop(fjt), pvl)
                    else:
                        emit_pvl(ic, jt, p_tiles, pvl)
                    if jt < 4:
                        # one i-row-block per jt: keeps the psum-tag ring and
                        # the DVE copy chain from parking >4 PE instructions
                        # (which would stall the whole in-order PE stream)
                        emit_outproj_it(ic - 1, osb, yps, jt)
                    if jt == 4 and ic < 3:
                        # sc(ic+1) q-features, needed by ic(ic+1)'s scores
                        emit_aqk_pair(ic + 1, (0, 1), tag="srot1")
                pvl_prev = pvl
            osb = emit_norm(3, pvl_prev, final=True)
            yps = [pvl_prev[0], pvl_prev[1], pvl_prev[2], pvl_prev[3],
                   s_fin[0][:, 0:512], s_fin[0][:, 512:1024],
                   s_fin[1][:, 0:512], s_fin[1][:, 512:1024]]
            for it in range(4):
                emit_outproj_it(3, osb, yps, it, final=True)

    nc.compile()
    return nc


def _prep_inputs(x, w_qkv, w_out, b_out, void_q, void_k, void_v,
                 attention_trace, temperature_factor):
    """Host-side sharding / layout prep. Returns in_maps for 8 cores."""
    temp = np.maximum(1.0 + np.abs(attention_trace) * temperature_factor,
                      1.0).reshape(HEADS).astype(np.float32)
    scale = (DIM ** -0.5) / temp                       # [16] per head
    # split the score scale between q (fixed 1/4) and k (4*scale_h) so both
    # fp8 operands stay in e4m3's normal range
    kcol_scale = np.repeat(4.0 * scale, D)             # [1024]
    wq_scaled = (w_qkv[:, 0:DIM] * 0.25).astype(np.float32)
    wk_scaled = (w_qkv[:, DIM:2 * DIM] * kcol_scale[None, :]).astype(np.float32)
    wv_full = w_qkv[:, 2 * DIM:3 * DIM]
    vk = (void_k.reshape(HEADS, D) * (4.0 * scale)[:, None]).astype(np.float32)
    vv = void_v.reshape(HEADS, D)

    ebias = np.zeros((P, 1), np.float32)
    ebias[1:, 0] = -100.0

    in_maps = []
    for core in range(8):
        b, hg = divmod(core, 4)
        h0 = hg * HPC
        cs = slice(h0 * D, (h0 + HPC) * D)             # 256 feature cols
        in_maps.append({
            "xT": np.ascontiguousarray(x[b].T),
            "wqkv": np.ascontiguousarray(
                np.concatenate([wq_scaled[:, cs], wk_scaled[:, cs],
                                wv_full[:, cs]], axis=1)),
            "wout": np.ascontiguousarray(w_out[cs, :]),
            # voidk rows: partition p = hh*64 + d, col = pair
            "voidk": np.ascontiguousarray(
                vk[h0:h0 + HPC].reshape(2, 2, D)       # [pair, hh, d]
                .transpose(1, 2, 0).reshape(P, 2)),
            "voidv": np.ascontiguousarray(vv[h0:h0 + HPC].reshape(1, 256)),
            "ebias_in": ebias,
        })
    return in_maps


def _run(in_maps, trace=False):
    from concourse import bass_utils
    if "nc" not in _cache:
        _cache["nc"] = _build()
    return bass_utils.run_bass_kernel_spmd(
        _cache["nc"], in_maps, core_ids=list(range(8)), trace=trace)


def kernel(x, w_qkv, w_out, b_out, void_q, void_k, void_v,
           attention_trace, temperature_factor):
    args = [np.asarray(a, dtype=np.float32) for a in
            (x, w_qkv, w_out, b_out, void_q, void_k, void_v,
             attention_trace, temperature_factor)]
    in_maps = _prep_inputs(*args)
    res = _run(in_maps)
    out = np.zeros((B, N, DIM), np.float32)
    for core in range(8):
        b = core // 4
        out[b] += res.results[core]["y"]
    out += args[3][None, None, :]                      # b_out
    return out


# revision 63
# speedup vs baseline: 1.0256x; 1.0046x over previous
"""BlanchotianAttention TRN2 kernel: 8 NeuronCores, data-parallel over batch (2)
x tensor-parallel over heads (4 heads/core). 206.6us cost-model exec
(260.1us baseline), rel err 1.22e-2 (gate 2e-2).

Key techniques over the baseline:
  - scores matmuls in fp8e4 DoubleRow perf mode (0.5 cyc/row): Q and K are
    quantized to fp8 during the stage-A PSUM evacuation copies. The DoubleRow
    second k-tile slot is zero-filled (contraction depth is only d=64). The
    dim^-0.5/temp score scale is split between q (x0.25) and k (x 4*scale_h)
    so both fp8 operands sit in e4m3's normal range.
  - exp output P and V_aug are bf16: same matmul rate as f32r, half the SBUF,
    ~2e-3 error contribution.
  - inputs DMA directly into per-chunk float32r SBUF tiles (f32r is
    bit-identical to f32; rounding happens inside the PE). Per-chunk tiles
    matter: the Tile framework tracks dependencies per tile, so one big tile
    would make the first matmul wait for every input DMA.
  - DMA arrival order is tuned for the serial DMA pipe: interleaved
    (w_qk[ko], xT[ko, sc0]) pairs feed the first stage-A accumulation at pipe
    speed; later chunks stream behind, ko-major stage A tracks arrival. No
    input DMAs on the scalar queue: their ring-full waits clog ACT.SEQ (4-deep
    wait queue) and stall the exps behind them. First exp at ~17.5us vs ~44us.
  - the two srot PSUM tags are the scarce resource (scores need one rotation
    per j-tile per pair); window stage-A accumulators pack 2 features or
    2 v-blocks per [128,1024] tile to minimize ring insertions. PSUM
    start=True resets a whole 2KB bank, so no two accumulation groups may
    share one (av accumulators use one group per bank).
  - outproj writes its y psum in place into the drained pvl accumulators of
    the previous i-chunk; fresh psum tiles would entangle slot rings via the
    pool's LIFO allocator and serialize the tail. The next chunk's pvl
    allocation is deferred to jt5 (p tiles wait in the 6-deep pexp ring) so
    the banks hand over cleanly. One outproj row-block per j-tile keeps
    blocked PE instructions under the 4-deep park limit, past which the
    whole in-order PE stream stalls.
  - final outproj reuses the dead jt16 score tiles as y psum, streams each
    512-col half to DRAM as its copy lands, and puts half the copies on ACT
    (idle after the last exp; can read PSUM, which GPSIMD cannot).

Layout (per core, batch b, head-group hg = heads h0..h0+3):
  - stage A-qk: psum = w.T @ xT -> QT/KT [d, seq] head-pair tiles, evacuated
    to fp8 qt8/kt8 (partitions 0:64 even head of pair, 64:128 odd head).
  - stage A-v: V = x @ w_v in [seq, d] layout, augmented per head with a ones
    block: V_aug[j, h*128 : h*128+128] = [v_h (64) | ones (64)] (f32r).
  - stage B/C per (512-wide i-chunk, head pair, 128-wide j-tile):
    S^T = K @ Q^T via one DoubleRow matmul per head,
    P = exp(S^T) on ACT (one [128,1024] activation covers both heads),
    PV+l fused f32r: matmul(lhsT=[v_h | ones], rhs=P) accumulates attn@v in
    PSUM rows 0-63 and the softmax denominator (broadcast) in rows 64-127.
  - void token: the void QUERY row is dropped by the reference so it is never
    computed. The void KEY/VALUE occupy j=2048 inside j-tile 16, zero-padded
    to 128 rows; a per-partition exp bias of -100 on that tile zeroes the pad
    rows' contributions.
  - normalize: reciprocal of the l rows + cross-base multiply -> O^T pair tile.
  - stage D: y_partial = O_norm @ w_out_shard; host sums partials over the 4
    head-group cores of each batch (+ b_out).
"""
import sys

sys.path.insert(0, "/opt/trn_rl_repo")

import numpy as np

DIM, HEADS, B, N = 1024, 16, 2, 2048
D = DIM // HEADS          # 64
HPC = HEADS // 4          # heads per core = 4
NJT = 17                  # j tiles (16 full + void/pad tile)
P = 128

_cache = {}
DEBUG = False


def _build():
    import concourse.bass as bass
    import concourse.mybir as mybir
    import concourse.tile as tile
    from concourse import bacc

    F32 = mybir.dt.float32
    F32R = mybir.dt.float32r
    BF16 = mybir.dt.bfloat16
    FP8 = mybir.dt.float8e4
    DR = mybir.MatmulPerfMode.DoubleRow
    Exp = mybir.ActivationFunctionType.Exp
    Rcp = mybir.ActivationFunctionType.Reciprocal

    nc = bacc.Bacc("TRN2", target_bir_lowering=False, debug=False)
    xT = nc.dram_tensor("xT", [DIM, N], F32R, kind="ExternalInput").ap()
    wqkv = nc.dram_tensor("wqkv", [DIM, 768], F32R, kind="ExternalInput").ap()
    wout = nc.dram_tensor("wout", [256, DIM], F32R, kind="ExternalInput").ap()
    voidk = nc.dram_tensor("voidk", [P, 2], F32, kind="ExternalInput").ap()
    voidv = nc.dram_tensor("voidv", [1, 256], F32, kind="ExternalInput").ap()
    ebias_in = nc.dram_tensor("ebias_in", [P, 1], F32, kind="ExternalInput").ap()
    y = nc.dram_tensor("y", [N, DIM], F32, kind="ExternalOutput").ap()
    if DEBUG:
        dbg_qt = nc.dram_tensor("dbg_qt", [P, 2, 2, N], FP8,
                                kind="ExternalOutput").ap()
        dbg_kt = nc.dram_tensor("dbg_kt", [P, 2, 2, NJT * P], FP8,
                                kind="ExternalOutput").ap()
        dbg_va = nc.dram_tensor("dbg_va", [P, NJT, 512], BF16,
                                kind="ExternalOutput").ap()
        dbg_pvl = nc.dram_tensor("dbg_pvl", [P, 512], F32,
                                 kind="ExternalOutput").ap()

    KO = DIM // P  # 8 k-tiles

    with tile.TileContext(nc) as tc:
        with tc.tile_pool(name="persist", bufs=1) as pp, \
             tc.tile_pool(name="work", bufs=1) as wp, \
             tc.tile_pool(name="psum", bufs=1, space="PSUM") as ps, \
             tc.tile_pool(name="loadA", bufs=2) as lp:

            # ---- constants ----
            ones = pp.tile([P, D], F32)
            nc.vector.memset(ones[:], 1.0)
            ebias = pp.tile([P, 1], F32)

            # ---- persistent SBUF tensors ----
            # qt8/kt8: [part, pair, doublerow-ktile, seq]; partitions 0:64 =
            # even head of the pair, 64:128 = odd head; ktile slot 1 is zeroed.
            qt8 = pp.tile([P, 2, 2, N], FP8)
            kt8 = pp.tile([P, 2, 2, NJT * P], FP8)
            va = pp.tile([P, NJT, 512], BF16)          # V_aug per j-tile
            wq_t = [pp.tile([P, 512], F32R, name=f"wq_{ko}") for ko in range(KO)]
            wv_t = [pp.tile([P, 256], F32R, name=f"wv_{ko}") for ko in range(KO)]
            wout_t = [pp.tile([P, DIM], F32R, name=f"wout_{h}") for h in range(2)]
            xt_t = [[pp.tile([P, 512], F32R, name=f"xt_{ko}_{sc}")
                     for sc in range(4)] for ko in range(KO)]

            # DoubleRow slot-1 zeros: off the DMA critical path
            nc.vector.memset(qt8[:, :, 1, :], 0.0)
            nc.gpsimd.memset(kt8[:, :, 1, :], 0.0)

            # V_aug ones blocks (DVE, depends only on `ones`)
            for jt in range(NJT):
                nc.vector.tensor_copy(
                    va[:, jt, :].rearrange("p (h c) -> p h c", c=P)[:, :, D:P],
                    ones[:, None, :].to_broadcast([P, 4, D]))

            # ---- input DMA: serial DMA pipe => arrival order is everything.
            # Nothing on the scalar (ACT) queue: DMA issue instructions with
            # ring-full waits clog ACT.SEQ and stall the exps behind them.
            for ko in range(KO):
                nc.sync.dma_start(wq_t[ko][:], wqkv[ko * P:(ko + 1) * P, 0:512])
                nc.sync.dma_start(xt_t[ko][0][:],
                                  xT[ko * P:(ko + 1) * P, 0:512])
            for ko in range(KO):
                nc.sync.dma_start(xt_t[ko][1][:],
                                  xT[ko * P:(ko + 1) * P, 512:1024])
            for ko in range(KO):
                nc.sync.dma_start(wv_t[ko][:],
                                  wqkv[ko * P:(ko + 1) * P, 512:768])
            for ko in range(KO):
                nc.sync.dma_start(xt_t[ko][2][:],
                                  xT[ko * P:(ko + 1) * P, 1024:1536])
            nc.sync.dma_start(ebias[:], ebias_in)
            vkt = lp.tile([P, 2], F32, tag="stg", name="vkt")
            nc.sync.dma_start(vkt[:], voidk)
            vvt = lp.tile([1, 256], F32, tag="stg", name="vvt")
            nc.sync.dma_start(vvt[:], voidv)
            for ko in range(KO):
                nc.sync.dma_start(xt_t[ko][3][:],
                                  xT[ko * P:(ko + 1) * P, 1536:2048])
            for h in range(2):
                nc.sync.dma_start(wout_t[h][:], wout[h * P:(h + 1) * P, :])

            # ---- stage A emit helpers ----
            def emit_aqk_pair(sc, fts, tag="srot0"):
                # fts: (0,1) or (2,3). tag may be a single srot tag (both
                # features packed in ONE [128,1024] tile = one ring slot) or
                # a (tagA, tagB) pair of [128,512] single-bank tiles (used in
                # early ic0 when the pvl banks are still free). ko-major so
                # PSUM accumulation tracks DMA arrival.
                if isinstance(tag, tuple):
                    accs = [ps.tile([P, 512], F32, tag=t,
                                    name=f"aqk_{sc}_{fts[i]}")
                            for i, t in enumerate(tag)]
                    regions = [accs[0][:, 0:512], accs[1][:, 0:512]]
                else:
                    acc = ps.tile([P, 1024], F32, tag=tag,
                                  name=f"aqk_{sc}_{fts[0]}")
                    regions = [acc[:, 0:512], acc[:, 512:1024]]
                for ko in range(KO):
                    for i, ft in enumerate(fts):
                        nc.tensor.matmul(
                            regions[i],
                            wq_t[ko][:, ft * P:(ft + 1) * P],
                            xt_t[ko][sc][:],
                            start=(ko == 0), stop=(ko == KO - 1),
                        )
                for i, ft in enumerate(fts):
                    dst = qt8 if ft < 2 else kt8
                    nc.vector.tensor_copy(
                        dst[:, ft % 2, 0, sc * 512:(sc + 1) * 512],
                        regions[i])

            def emit_av2(q, half, tag):
                # 2 avs with one accumulation group per 2KB bank: PSUM
                # start=True resets the whole bank, so two groups must not
                # share one. tag: single srot tag ([128,1024], groups in its
                # two banks) or a (tagA, tagB) pair of single-bank tiles.
                stis = (2 * half, 2 * half + 1)
                if isinstance(tag, tuple):
                    accs = [ps.tile([P, 512], F32, tag=t,
                                    name=f"avw_{q}_{half}_{i}")
                            for i, t in enumerate(tag)]
                    regions = [accs[0][:, 0:256], accs[1][:, 0:256]]
                else:
                    acc = ps.tile([P, 1024], F32, tag=tag,
                                  name=f"avw_{q}_{half}")
                    regions = [acc[:, 0:256], acc[:, 512:768]]
                for ko in range(KO):
                    for i, sti in enumerate(stis):
                        nc.tensor.matmul(
                            regions[i],
                            xt_t[ko][q][:, sti * P:(sti + 1) * P],
                            wv_t[ko][:],
                            start=(ko == 0), stop=(ko == KO - 1),
                        )
                for i, sti in enumerate(stis):
                    st = 4 * q + sti
                    nc.vector.tensor_copy(
                        va[:, st, :].rearrange("p (h c) -> p h c", c=P)[:, :, 0:D],
                        regions[i].rearrange("p (h c) -> p h c", c=D))

            def emit_void_setup():
                # void k column + pad zeros + V_aug void row
                nc.vector.tensor_copy(kt8[:, :, 0, 2048:2049], vkt[:])
                nc.vector.memset(kt8[:, :, 0, 2049:NJT * P], 0.0)
                va16 = va[:, 16, :]
                nc.vector.memset(
                    va16.rearrange("p (h c) -> p h c", c=P)[:, :, 0:D]
                    .bitcast(F32), 0.0)
                nc.vector.tensor_copy(
                    va16.rearrange("p (h c) -> p h c", c=P)[0:1, :, 0:D],
                    vvt[:].rearrange("p (h c) -> p h c", c=D))

            # ---- stage B/C/D emit helpers ----
            def emit_scores_pair(ic, jt, pair):
                isl = slice(ic * 512, (ic + 1) * 512)
                jsl = slice(jt * P, (jt + 1) * P)
                s_pair = ps.tile([P, 1024], F32, tag=f"srot{pair}",
                                 name=f"s_{ic}_{jt}_{pair}")
                for hh in range(2):
                    nc.tensor.matmul(
                        s_pair[:, hh * 512:(hh + 1) * 512],
                        kt8[hh * D:(hh + 1) * D, pair, :, jsl],
                        qt8[hh * D:(hh + 1) * D, pair, :, isl],
                        start=True, stop=True, perf_mode=DR)
                return s_pair

            def emit_scores(ic, jt):
                return [emit_scores_pair(ic, jt, pair) for pair in range(2)]

            def emit_exps(ic, jt, s_cur):
                p_tiles = []
                for pair in range(2):
                    p_pair = wp.tile([P, 1024], BF16, tag=f"pexp{pair}",
                                     bufs=6,
                                     name=f"p_{ic}_{jt}_{pair}")
                    if jt == 16:
                        nc.scalar.activation(p_pair[:], s_cur[pair][:], Exp,
                                             bias=ebias[:])
                    else:
                        nc.scalar.activation(p_pair[:], s_cur[pair][:], Exp)
                    p_tiles.append(p_pair)
                return p_tiles

            def emit_pvl(ic, jt, p_tiles, pvl):
                for pair in range(2):
                    for hh in range(2):
                        h = 2 * pair + hh
                        nc.tensor.matmul(
                            pvl[h][:],
                            va[:, jt, h * P:(h + 1) * P],
                            p_tiles[pair][:, hh * 512:(hh + 1) * 512],
                            start=(jt == 0), stop=(jt == 16),
                        )

            def emit_exp_pvl(ic, jt, s_cur, pvl, nxt, mid=None):
                """exp(jt) ; scores(nxt) ; [mid()] ; pvl(jt)."""
                p_tiles = emit_exps(ic, jt, s_cur)
                s_nxt = emit_scores(*nxt) if nxt is not None else None
                if mid is not None:
                    mid()
                emit_pvl(ic, jt, p_tiles, pvl)
                return s_nxt

            def emit_norm(ic, pvl, final=False):
                """normalize pvl -> osb SBUF tiles."""
                osb = [wp.tile([P, 512], F32R,
                               tag=f"osbf{pair}" if final else f"osb{pair}",
                               bufs=2, name=f"osb{pair}_{ic}")
                       for pair in range(2)]
                rsbs = []
                for h in range(4):
                    r_sb = lp.tile([P, 512], F32, tag="rsbf" if final else "rsb",
                                   bufs=4 if final else 2,
                                   name=f"rsb_{ic}_{h}")
                    rsbs.append(r_sb)
                    nc.vector.reciprocal(r_sb[D:P, :], pvl[h][D:P, :])
                    if not final:
                        pair, hh = divmod(h, 2)
                        nc.vector.tensor_tensor(
                            osb[pair][hh * D:(hh + 1) * D, :],
                            pvl[h][0:D, :], r_sb[D:P, :],
                            mybir.AluOpType.mult)
                if final:
                    # all recips first, then the mults
                    for h in range(4):
                        pair, hh = divmod(h, 2)
                        nc.vector.tensor_tensor(
                            osb[pair][hh * D:(hh + 1) * D, :],
                            pvl[h][0:D, :], rsbs[h][D:P, :],
                            mybir.AluOpType.mult)
                return osb

            def emit_outproj_it(ic, osb, yps, it, final=False):
                # yps are DEAD psum tiles (the drained pvl accumulators / last
                # score tiles) written in place: allocating fresh psum tiles
                # would share slots with the next ic's pvl accumulators via
                # the pool's LIFO allocator and serialize the whole tail
                ysb = wp.tile([P, DIM], F32, tag="ysbf" if final else "ysb",
                              bufs=2 if final else 4,
                              name=f"ysb_{ic}_{it}")
                for oc in range(2):
                    yp = yps[it * 2 + oc]
                    for pair in range(2):
                        nc.tensor.matmul(
                            yp[:],
                            osb[pair][:, it * P:(it + 1) * P],
                            wout_t[pair][:, oc * 512:(oc + 1) * 512],
                            start=(pair == 0), stop=(pair == 1),
                        )
                    if final and oc == 1:
                        # ACT is idle after the last exp and can read PSUM
                        nc.scalar.copy(ysb[:, oc * 512:(oc + 1) * 512], yp[:])
                    else:
                        nc.vector.tensor_copy(ysb[:, oc * 512:(oc + 1) * 512],
                                              yp[:])
                    if final:
                        # don't wait for the other half: stream each 512-col
                        # half as soon as its copy lands
                        nc.sync.dma_start(
                            y[ic * 512 + it * P: ic * 512 + (it + 1) * P,
                              oc * 512:(oc + 1) * 512],
                            ysb[:, oc * 512:(oc + 1) * 512])
                if not final:
                    nc.sync.dma_start(
                        y[ic * 512 + it * P: ic * 512 + (it + 1) * P, :],
                        ysb[:])

            def alloc_pvl(ic):
                return [ps.tile([P, 512], F32, tag=f"pvl{h}", name=f"pvl{h}_{ic}")
                        for h in range(4)]

            # ---- main schedule ----
            # sc0's 4 q/k features run upfront, ko-major (tracking DMA
            # arrival), packed into the two srot tiles' halves so both score
            # pairs can fire back-to-back; ic1..3 plain.
            # pvl allocation is always deferred to jt5 so the previous
            # owner of the banks (outproj writes / early-window accs) is done
            flush_plan = {5: (0, 1, 2), 6: (3, 4, 5), 7: (6, 7)}
            accp = [ps.tile([P, 1024], F32, tag=f"srot{i}", name=f"a0_p{i}")
                    for i in range(2)]
            for ko in range(KO):
                # (ft, pair-acc, col offset): pair0 = ft0|ft2, pair1 = ft1|ft3
                for ft, pair in ((0, 0), (2, 0), (1, 1), (3, 1)):
                    off = 0 if ft < 2 else 512
                    nc.tensor.matmul(
                        accp[pair][:, off:off + 512],
                        wq_t[ko][:, ft * P:(ft + 1) * P],
                        xt_t[ko][0][:],
                        start=(ko == 0), stop=(ko == KO - 1),
                    )
            for pair in range(2):
                nc.vector.tensor_copy(qt8[:, pair, 0, 0:512],
                                      accp[pair][:, 0:512])
                nc.vector.tensor_copy(kt8[:, pair, 0, 0:512],
                                      accp[pair][:, 512:1024])
            s00_p0 = emit_scores_pair(0, 0, 0)
            s00_p1 = emit_scores_pair(0, 0, 1)
            s_cur = [s00_p0, s00_p1]
            # ic0 flat jt loop. pvl(0) allocation is deferred to jt5 (p tiles
            # wait in the pexp ring), so during jts 0-4 the four pvl banks are
            # free: the early window accumulators (avs for sc0/sc1-keys,
            # sc1 q-features) live there instead of stealing srot ring slots
            # from the score pipeline.
            hooks = {
                1: lambda: emit_av2(0, 0, ("pvl0", "pvl1")),
                2: lambda: emit_av2(0, 1, ("pvl2", "pvl3")),
                3: lambda: emit_aqk_pair(1, (2, 3), ("pvl0", "pvl1")),
                4: lambda: emit_aqk_pair(1, (0, 1), ("pvl2", "pvl3")),
                5: lambda: emit_av2(1, 0, "srot1"),
                6: lambda: emit_av2(1, 1, "srot0"),
                7: lambda: (emit_aqk_pair(2, (2, 3), "srot0"),
                            emit_av2(2, 0, "srot1")),
                8: lambda: emit_void_setup(),
                9: lambda: emit_av2(2, 1, "srot0"),
                11: lambda: (emit_aqk_pair(3, (2, 3), "srot0"),
                             emit_av2(3, 0, "srot1")),
                13: lambda: emit_av2(3, 1, "srot0"),
            }
            held = {}
            pvl = None
            for jt in range(NJT):
                p_tiles = emit_exps(0, jt, s_cur)
                if jt in hooks:
                    hooks[jt]()
                nxt = (0, jt + 1) if jt < NJT - 1 else (1, 0)
                s_cur = emit_scores(*nxt)
                if jt < 8:
                    held[jt] = p_tiles
                    if jt == 5:
                        pvl = alloc_pvl(0)
                    for fjt in flush_plan.get(jt, ()):
                        emit_pvl(0, fjt, held.pop(fjt), pvl)
                else:
                    emit_pvl(0, jt, p_tiles, pvl)

            if DEBUG:
                nc.sync.dma_start(dbg_qt, qt8[:])
                nc.sync.dma_start(dbg_kt, kt8[:])
                nc.sync.dma_start(dbg_va, va[:])
                dbg_sb = wp.tile([P, DIM], F32, tag="ysb", bufs=4,
                                 name="dbg_sb")
                nc.vector.tensor_copy(dbg_sb[:, 0:512], pvl[0][:])
                nc.sync.dma_start(dbg_pvl, dbg_sb[:, 0:512])
            pvl_prev = pvl
            s_fin = None
            # outproj(ic-1) writes its y psum in place into the drained
            # pvl(ic-1) tiles; pvl(ic)'s tag slots alias those banks, so its
            # allocation + first jts' matmuls are deferred past the outproj
            # writes (p tiles wait in the pexp ring)
            for ic in range(1, 4):
                osb = emit_norm(ic - 1, pvl_prev)
                yps = [pvl_prev[k % 4] for k in range(8)]
                pvl = None
                held = {}
                for jt in range(NJT):
                    if jt == NJT - 1:
                        nxt = (ic + 1, 0) if ic < 3 else None
                    else:
                        nxt = (ic, jt + 1)
                    if ic == 3 and jt == NJT - 1:
                        s_fin = s_cur          # jt16 score tiles, dead after
                    p_tiles = emit_exps(ic, jt, s_cur)
                    s_cur = emit_scores(*nxt) if nxt is not None else None
                    if jt < 8:
                        held[jt] = p_tiles
                        if jt == 5:
                            pvl = alloc_pvl(ic)
                        for fjt in flush_plan.get(jt, ()):
                            emit_pvl(ic, fjt, held.pop(fjt), pvl)
                    else:
                        emit_pvl(ic, jt, p_tiles, pvl)
                    if jt < 4:
                        # one i-row-block per jt: keeps the psum-tag ring and
                        # the DVE copy chain from parking >4 PE instructions
                        # (which would stall the whole in-order PE stream)
                        emit_outproj_it(ic - 1, osb, yps, jt)
                    if jt == 3 and ic < 3:
                        # sc(ic+1) q-features, needed by ic(ic+1)'s scores.
                        # outproj its 0/2 (jts 0/2) are done with the
                        # pvl0/pvl1 banks by now and the next pvl allocation
                        # only happens at jt5, so these accs slot in between
                        # instead of stealing a srot ring rotation
                        emit_aqk_pair(ic + 1, (0, 1), ("pvl0", "pvl1"))
                pvl_prev = pvl
            osb = emit_norm(3, pvl_prev, final=True)
            yps = [pvl_prev[0], pvl_prev[1], pvl_prev[2], pvl_prev[3],
                   s_fin[0][:, 0:512], s_fin[0][:, 512:1024],
                   s_fin[1][:, 0:512], s_fin[1][:, 512:1024]]
            for it in range(4):
                emit_outproj_it(3, osb, yps, it, final=True)

    nc.compile()
    return nc


def _prep_inputs(x, w_qkv, w_out, b_out, void_q, void_k, void_v,
                 attention_trace, temperature_factor):
    """Host-side sharding / layout prep. Returns in_maps for 8 cores."""
    temp = np.maximum(1.0 + np.abs(attention_trace) * temperature_factor,
                      1.0).reshape(HEADS).astype(np.float32)
    scale = (DIM ** -0.5) / temp                       # [16] per head
    # split the score scale between q (fixed 1/4) and k (4*scale_h) so both
    # fp8 operands stay in e4m3's normal range
    kcol_scale = np.repeat(4.0 * scale, D)             # [1024]
    wq_scaled = (w_qkv[:, 0:DIM] * 0.25).astype(np.float32)
    wk_scaled = (w_qkv[:, DIM:2 * DIM] * kcol_scale[None, :]).astype(np.float32)
    wv_full = w_qkv[:, 2 * DIM:3 * DIM]
    vk = (void_k.reshape(HEADS, D) * (4.0 * scale)[:, None]).astype(np.float32)
    vv = void_v.reshape(HEADS, D)

    ebias = np.zeros((P, 1), np.float32)
    ebias[1:, 0] = -100.0

    in_maps = []
    for core in range(8):
        b, hg = divmod(core, 4)
        h0 = hg * HPC
        cs = slice(h0 * D, (h0 + HPC) * D)             # 256 feature cols
        in_maps.append({
            "xT": np.ascontiguousarray(x[b].T),
            "wqkv": np.ascontiguousarray(
                np.concatenate([wq_scaled[:, cs], wk_scaled[:, cs],
                                wv_full[:, cs]], axis=1)),
            "wout": np.ascontiguousarray(w_out[cs, :]),
            # voidk rows: partition p = hh*64 + d, col = pair
            "voidk": np.ascontiguousarray(
                vk[h0:h0 + HPC].reshape(2, 2, D)       # [pair, hh, d]
                .transpose(1, 2, 0).reshape(P, 2)),
            "voidv": np.ascontiguousarray(vv[h0:h0 + HPC].reshape(1, 256)),
            "ebias_in": ebias,
        })
    return in_maps


def _run(in_maps, trace=False):
    from concourse import bass_utils
    if "nc" not in _cache:
        _cache["nc"] = _build()
    return bass_utils.run_bass_kernel_spmd(
        _cache["nc"], in_maps, core_ids=list(range(8)), trace=trace)


def kernel(x, w_qkv, w_out, b_out, void_q, void_k, void_v,
           attention_trace, temperature_factor):
    args = [np.asarray(a, dtype=np.float32) for a in
            (x, w_qkv, w_out, b_out, void_q, void_k, void_v,
             attention_trace, temperature_factor)]
    in_maps = _prep_inputs(*args)
    res = _run(in_maps)
    out = np.zeros((B, N, DIM), np.float32)
    for core in range(8):
        b = core // 4
        out[b] += res.results[core]["y"]
    out += args[3][None, None, :]                      # b_out
    return out
